# revision 1
# baseline (speedup 1.0000x reference)
"""Trainium2 Bass kernel for a RoPE causal-attention layer.

Problem (hardcoded): B=2, T=2048, DIM=1024, H=16 heads, Dh=64, fp32.
  qkv = x @ qkv_w.T + qkv_b ; rope(q), rope(k) ; causal softmax(q k^T / sqrt(Dh)) @ v
  out = ctx @ out_w.T + out_b

Sharding: tensor-parallel over heads — each of the 8 cores owns 2 heads
(qkv_w row-sharded, out_w column-sharded); per-core partial outputs are
summed on the host.

Per-core kernel layout notes:
  * Activations live transposed ([feature, token]) so every matmul
    contracts over partitions: xT [1024, 4096] -> qT/kT/vT [128, 4096].
  * All matmul operands are float32r (fp32 storage, single-pass PE at 4x
    the fp32 matmul rate, ~TF32 operand rounding); PSUM accumulation is
    full fp32.  End-to-end relative error vs the fp32 reference: 2e-4.
  * RoPE pair-interleave is folded into the q/k weight rows on the host
    (de-interleave permutation), making the on-device rotation
    q_rot = q*C + swap_halves(q)*S with contiguous halves; swap_halves is
    one PE permutation matmul, the rest is 3 DVE elementwise ops.
  * Scores are computed TRANSPOSED ([s, t] tiles) and the PV matmul is
    transposed as well: ctx^T(+denominator row) = v_aug^T @ exp(s^T) -
    one K=128/N=512 matmul per s-block, no probability or context
    transposes anywhere, and the context lands pre-transposed for the
    output projection.
  * Softmax denominators come for free from a ones-column appended to V;
    the per-token divide broadcasts the reciprocal denominator row
    across partitions with a tiny K=1 ones-matmul on the PE.
    exp() needs no max-subtraction: inputs are fixed-scale (|score|<~30).
  * Causality: only lower-triangular s-blocks are visited; diagonal
    blocks compute only the at/right-of-diagonal column range and mask
    just the diagonal 128-sub with a [128,128] triangle (exp(-inf)=0
    expressed as a multiplicative 0/1 mask after exp - exact).
  * Phases are software-interleaved at emission time (engines execute
    their instruction streams in order): batch-1 projections overlap
    batch-0 attention, output projection overlaps batch-1 attention; the
    two heads' score/exp/PV chains are interleaved 2-deep so the PE
    always has an independent matmul between a score and its dependent
    PV.
"""

import sys

if "/opt/trn_rl_repo" not in sys.path:
    sys.path.insert(0, "/opt/trn_rl_repo")

import numpy as np

import concourse.bass as bass
import concourse.tile as tile
from concourse import mybir
from concourse.vector_clock import ScopedClock, VectorClock

B, T, DIM = 2, 2048, 1024
H, Dh = 16, 64
NCORES = 8
HPC = H // NCORES          # heads per core
NT = B * T                 # 4096 tokens
RPC = HPC * Dh             # 128 rows per core for each of q/k/v
NQC = NT // 512            # 8 column chunks for projections
SCALE = Dh ** -0.5

F32 = mybir.dt.float32


def _patch_tile_drain():
    """This container's walrus build allows at most ONE semaphore wait per
    instruction (setupSyncWait rejects more).  Two fixes:
      1. Tile's end-of-kernel drain accumulates one wait per live
         semaphore - split into one drain per semaphore.
      2. Any scheduled instruction that received >1 sem waits in stage 1B
         gets its extra waits hoisted onto same-engine NoOps emitted just
         before it during lowering."""
    if getattr(tile.TileContext, "_drain_patched", False):
        return

    def patched(self, tick_clock, wait_clock):
        vec = list(tick_clock.global_clock)
        nz = [(i, t) for i, t in enumerate(vec) if t > 0] or [(0, 0)]
        for i, t in nz:
            cv = [0] * len(vec)
            cv[i] = t
            d = self.nc.sync.drain()
            wait_clock.add_sem_waits(d.ins, ScopedClock({None: VectorClock(cv)}))
        self.nc.all_engine_barrier()
        popped = self.nc._tile_sem_poison_stack.pop()
        assert popped is self._sem_poison
        self.nc.clear_and_free_semaphores(list(self.sems.allocated().values()))
        self.nc.all_engine_barrier()

    tile.TileContext._drain_and_barrier = patched

    orig_cal = tile.TileContext._commit_and_lower

    def patched_cal(self, inst, original_block, old_bb_map, bb_to_exit_bb):
        si = getattr(inst, "sync_info", None)
        eng = getattr(inst, "engine", None)
        if si is not None and si.on_wait and eng in self.nc.engines:
            waits = list(si.on_wait)
            # Matmult can't carry embedded waits at all in this walrus
            # build (fp32/fp32r lowering uses the LDW sync slots); other
            # instructions can carry exactly one.
            keep = 1
            if len(waits) > keep:
                for w in waits[: len(waits) - keep]:
                    nop = self.nc.engines[eng].nop(nofuse=True)
                    nop.ins.sync_info = mybir.SyncInfo(on_wait=[w], on_update=[])
                inst.sync_info = mybir.SyncInfo(
                    on_wait=waits[len(waits) - keep:],
                    on_update=list(si.on_update or []),
                )
        return orig_cal(self, inst, original_block, old_bb_map, bb_to_exit_bb)

    tile.TileContext._commit_and_lower = patched_cal
    tile.TileContext._drain_patched = True


def _rope_tables():
    """C, S [128, 2048] (f32) for the de-interleaved (halves) layout, rows
    duplicated for the 2 heads resident on a core.

    Reference rope on pair i of head_dim: angle_i(t) = t * inv_freq[(2i) % 32];
    de-interleaved row j (j<32: even element of pair j; j>=32: odd element of
    pair j-32):  q_rot = q*C + swap_halves(q)*S with
      C[j]    = cos(angle_{j%32}),  S[j] = -sin(angle_j) if j<32 else +sin(angle_{j-32}).
    """
    half = Dh // 2  # 32
    inv_freq = 1.0 / 10000.0 ** (np.arange(half, dtype=np.float64) / half)
    t = np.arange(T, dtype=np.float64)
    # pair i uses inv_freq[(2i) % 32]
    pair_freq = inv_freq[(2 * np.arange(half)) % half]        # [32]
    ang = np.outer(pair_freq, t)                              # [32, T]
    c32 = np.cos(ang)
    s32 = np.sin(ang)
    C64 = np.concatenate([c32, c32], axis=0)                  # [64, T]
    S64 = np.concatenate([-s32, s32], axis=0)                 # [64, T]
    C = np.concatenate([C64, C64], axis=0).astype(np.float32)  # [128, T]
    S = np.concatenate([S64, S64], axis=0).astype(np.float32)
    return np.ascontiguousarray(C), np.ascontiguousarray(S)


def _deinterleave_perm():
    """perm such that new[d] = old[perm[d]]: halves <- pair-interleaved."""
    p = np.empty(Dh, dtype=np.int64)
    p[: Dh // 2] = 2 * np.arange(Dh // 2)        # evens first
    p[Dh // 2:] = 2 * np.arange(Dh // 2) + 1     # odds second
    return p


def _swap_matrix():
    """[128,128] block-diag permutation: within each head's 64 rows, swap
    the two halves (rows 0..31 <-> 32..63)."""
    P64 = np.zeros((Dh, Dh), dtype=np.float32)
    half = Dh // 2
    P64[np.arange(half), half + np.arange(half)] = 1.0
    P64[half + np.arange(half), np.arange(half)] = 1.0
    M = np.zeros((RPC, RPC), dtype=np.float32)
    M[:Dh, :Dh] = P64
    M[Dh:, Dh:] = P64
    return M


def _tri_masks():
    """masks[k][i, j] = 1 if 128k + i <= j else 0 - the causal pattern of
    transposed-score diagonal blocks [s=128, t=512], k = s-block offset/128
    within the 512-wide t block."""
    i = np.arange(128)[:, None]
    j = np.arange(512)[None, :]
    return np.stack(
        [(128 * k + i <= j).astype(np.float32) for k in range(4)], axis=0
    )


def _build_nc(use_pad_mask: bool, reps: int = 1):
    _patch_tile_drain()
    nc = bass.Bass("TRN2", target_bir_lowering=False, debug=False,
                   num_devices=NCORES)

    # All matmul operands are float32r (same 4-byte storage as f32; the PE
    # runs them single-pass at 4x the fp32 matmul rate, ~TF32 accuracy).
    # PSUM accumulation stays full fp32.
    FR = mybir.dt.float32r

    xT = nc.dram_tensor("xT", [DIM, NT], FR, kind="ExternalInput")
    wq = nc.dram_tensor("wq", [128, DIM], FR, kind="ExternalInput")
    wk = nc.dram_tensor("wk", [128, DIM], FR, kind="ExternalInput")
    wv = nc.dram_tensor("wv", [128, DIM], FR, kind="ExternalInput")
    bq = nc.dram_tensor("bq", [RPC, 1], F32, kind="ExternalInput")
    bk = nc.dram_tensor("bk", [RPC, 1], F32, kind="ExternalInput")
    bv = nc.dram_tensor("bv", [RPC, 1], F32, kind="ExternalInput")
    ropec = nc.dram_tensor("ropec", [RPC, T], FR, kind="ExternalInput")
    ropes = nc.dram_tensor("ropes", [RPC, T], FR, kind="ExternalInput")
    swapm = nc.dram_tensor("swapm", [RPC, RPC], FR, kind="ExternalInput")
    ident = nc.dram_tensor("ident", [128, 128], FR, kind="ExternalInput")
    masks = nc.dram_tensor("masks", [4, 128, 512], FR, kind="ExternalInput")
    wo = nc.dram_tensor("wo", [RPC, DIM], FR, kind="ExternalInput")
    ones64 = nc.dram_tensor("ones64", [1, Dh], FR, kind="ExternalInput")
    if use_pad_mask:
        padv = nc.dram_tensor("padv", [B, 128, T // 128], F32,
                              kind="ExternalInput")
    outT = nc.dram_tensor("outT", [DIM, NT], F32, kind="ExternalOutput")

    EXP = mybir.ActivationFunctionType.Exp
    IDN = mybir.ActivationFunctionType.Identity
    CPY = mybir.ActivationFunctionType.Copy

    with tile.TileContext(nc) as tc:
        with (
            tc.tile_pool(name="consts", bufs=1) as consts,
            tc.tile_pool(name="persist", bufs=1) as persist,
            tc.tile_pool(name="xpool", bufs=14) as xpool,
            tc.tile_pool(name="qkvtmp", bufs=4) as qkvtmp,
            tc.tile_pool(name="ropetmp", bufs=4) as ropetmp,
            tc.tile_pool(name="exppool", bufs=8) as exppool,
            tc.tile_pool(name="normtmp", bufs=4) as normtmp,
            tc.tile_pool(name="outev", bufs=4) as outev,
            tc.tile_pool(name="drampool", bufs=4, space="DRAM") as drampool,
            tc.tile_pool(name="psA", bufs=4, space="PSUM") as psA,
            tc.tile_pool(name="ctxps", bufs=3, space="PSUM") as ctxps,
            tc.tile_pool(name="tps", bufs=1, space="PSUM") as tps,
        ):
            # ---- constants: ordered so Phase A can start ASAP -------------
            # (engines run their DMA streams in program order - weights and
            # rope tables first, the B/C-phase constants last)
            wq_s = consts.tile([128, DIM], FR, tag="wq")
            wk_s = consts.tile([128, DIM], FR, tag="wk")
            wv_s = consts.tile([128, DIM], FR, tag="wv")
            nc.sync.dma_start(out=wq_s[:], in_=wq[:])
            nc.gpsimd.dma_start(out=wk_s[:], in_=wk[:])
            nc.gpsimd.dma_start(out=wv_s[:], in_=wv[:])
            bq_s = consts.tile([RPC, 1], F32, tag="bq")
            bk_s = consts.tile([RPC, 1], F32, tag="bk")
            bv_s = consts.tile([RPC, 1], F32, tag="bv")
            nc.sync.dma_start(out=bq_s[:], in_=bq[:])
            nc.gpsimd.dma_start(out=bk_s[:], in_=bk[:])
            nc.gpsimd.dma_start(out=bv_s[:], in_=bv[:])
            swap_s = consts.tile([RPC, RPC], FR, tag="swapm")
            id_s = consts.tile([128, 128], FR, tag="ident")
            ones_s = consts.tile([1, Dh], FR, tag="ones64")
            nc.gpsimd.dma_start(out=swap_s[:], in_=swapm[:])
            nc.gpsimd.dma_start(out=id_s[:], in_=ident[:])
            nc.gpsimd.dma_start(out=ones_s[:], in_=ones64[:])
            ropec_s = consts.tile([RPC, T], FR, tag="ropec")
            ropes_s = consts.tile([RPC, T], FR, tag="ropes")
            mask_s = consts.tile([128, 4 * 512], FR, tag="masks")
            wo_s = consts.tile([RPC, DIM], FR, tag="wo")

            def emit_late_consts():
                # SWDGE so these do not queue ahead of the x-tile loads on
                # the HWDGE rings; must still be emitted before any
                # consumer (Tile dependencies follow program order).
                nc.gpsimd.dma_start(out=ropec_s[:], in_=ropec[:])
                nc.gpsimd.dma_start(out=ropes_s[:], in_=ropes[:])
                for k in range(4):
                    nc.gpsimd.dma_start(out=mask_s[:, k * 512:(k + 1) * 512],
                                        in_=masks[k])
                nc.gpsimd.dma_start(out=wo_s[:], in_=wo[:])

            if use_pad_mask:
                pad_s = consts.tile([128, B * (T // 128)], F32, tag="padv")
                for b in range(B):
                    nc.sync.dma_start(
                        out=pad_s[:, b * 16:(b + 1) * 16], in_=padv[b])

            # ---- persistent activations (per-512-chunk tiles so phases
            # can overlap at chunk granularity) ----------------------------
            qrot = [persist.tile([RPC, 512], FR, tag=f"qrot{n}",
                                 name=f"qrot{n}") for n in range(NQC)]
            krot = [persist.tile([RPC, 512], FR, tag=f"krot{n}",
                                 name=f"krot{n}") for n in range(NQC)]
            ctxt = [persist.tile([RPC, 512], FR, tag=f"ctxt{n}",
                                 name=f"ctxt{n}") for n in range(NQC)]
            vaug = {}
            for b in range(B):
                va = persist.tile([128, HPC * 16 * 65], FR, tag=f"vaug{b}")
                nc.vector.memset(va[:].bitcast(F32), 1.0)
                vaug[b] = va

            def emit_a_chunk(n):
                """QKV projection + RoPE + V transpose for one 512-token
                chunk."""
                t0 = n * 512
                xt = [xpool.tile([128, 512], FR, tag="xt",
                                 name=f"xt{n}_{kc}") for kc in range(8)]
                for kc in range(8):
                    nc.sync.dma_start(
                        out=xt[kc][:],
                        in_=xT[kc * 128:(kc + 1) * 128, t0:t0 + 512])
                tloc = t0 % T

                def project(w_s, b_s, dst_raw):
                    ps = psA.tile([128, 512], F32, tag="proj")
                    for kc in range(8):
                        nc.tensor.matmul(ps[:],
                                         w_s[:, kc * 128:(kc + 1) * 128],
                                         xt[kc][:], start=(kc == 0),
                                         stop=(kc == 7))
                    nc.scalar.activation(dst_raw[:], ps[:], IDN, bias=b_s[:])

                def rope(raw, dst):
                    sw = psA.tile([128, 512], F32, tag="proj", name="sw")
                    nc.tensor.matmul(sw[:], swap_s[:], raw[:], start=True,
                                     stop=True)
                    nc.vector.tensor_mul(dst, raw[:],
                                         ropec_s[:, tloc:tloc + 512])
                    rtmp = ropetmp.tile([128, 512], F32, tag="rtmp")
                    nc.vector.tensor_mul(rtmp[:], sw[:],
                                         ropes_s[:, tloc:tloc + 512])
                    nc.vector.tensor_add(dst, dst, rtmp[:])

                qraw = qkvtmp.tile([128, 512], FR, tag="qraw")
                project(wq_s, bq_s, qraw)
                kraw = qkvtmp.tile([128, 512], FR, tag="kraw")
                project(wk_s, bk_s, kraw)
                rope(qraw, qrot[n][:, :])
                vraw = qkvtmp.tile([128, 512], FR, tag="vraw")
                project(wv_s, bv_s, vraw)
                rope(kraw, krot[n][:, :])
                b = t0 // T
                for m in range(4):
                    blk = (tloc + m * 128) // 128
                    tp = tps.tile([128, 128], FR, tag="tp")
                    nc.tensor.transpose(tp[:], vraw[:, m * 128:(m + 1) * 128],
                                        id_s[:])
                    dst = vaug[b].rearrange("p (h c) -> p h c", h=HPC)[
                        :, :, blk * 65:blk * 65 + 64]
                    src_ = tp[:].rearrange("p (h d) -> p h d", h=HPC)
                    nc.scalar.activation(dst, src_, CPY)

            def emit_b_pair(b, i):
                """Flash attention for one (batch, 512-query-block), BOTH
                heads interleaved with a 2-deep software pipeline so the PE
                always has an independent score matmul between a score and
                the PV that depends on its exp.  Scores are transposed
                [s,t]; PV is transposed too: ctx^T (+denominator row) =
                vaug^T @ exp(scores^T)."""
                tq0 = b * T + i * 512
                nj = 4 * (i + 1)
                cps = {}
                exs = {}
                for h in range(HPC):
                    cps[h] = ctxps.tile([65, 512], F32, tag="cps",
                                        name=f"cps{b}{h}{i}")

                def emit_sc(h, j):
                    hh = h * Dh
                    kchunk = (b * T + j * 128) // 512
                    koff = (j * 128) % 512
                    klhs = krot[kchunk][hh:hh + Dh, koff:koff + 128]
                    qrhs = qrot[4 * b + i][hh:hh + Dh, :]
                    sc = psA.tile([128, 512], F32, tag="proj",
                                  name=f"sc{b}{h}{i}{j}")
                    ex = exppool.tile([128, 512], FR, tag="ex")
                    kdiag = j - (nj - 4)
                    if kdiag < 0:
                        nc.tensor.matmul(sc[:], klhs, qrhs,
                                         start=True, stop=True)
                        nc.scalar.activation(ex[:], sc[:], EXP)
                    else:
                        # diagonal block: compute from the diagonal
                        # rightward; the diagonal 128-sub gets the
                        # triangular mask, the rest needs none
                        cols = slice(kdiag * 128, 512)
                        dcols = slice(kdiag * 128, (kdiag + 1) * 128)
                        nc.tensor.matmul(sc[:, cols], klhs, qrhs[:, cols],
                                         start=True, stop=True)
                        nc.scalar.activation(ex[:, cols], sc[:, cols], EXP)
                        nc.vector.tensor_mul(ex[:, dcols], ex[:, dcols],
                                             mask_s[:, 0:128])
                    if use_pad_mask:
                        nc.vector.tensor_scalar_mul(
                            ex[:], ex[:],
                            pad_s[:, b * 16 + j:b * 16 + j + 1])
                    exs[(h, j)] = ex

                def emit_pv(h, j):
                    va = vaug[b][:, h * (16 * 65):(h + 1) * (16 * 65)]
                    kdiag = max(0, j - (nj - 4))
                    cols = slice(kdiag * 128, 512)
                    nc.tensor.matmul(cps[h][:, cols],
                                     va[:, j * 65:j * 65 + 65],
                                     exs.pop((h, j))[:, cols],
                                     start=(j == 0), stop=(j == nj - 1))

                for j in range(nj + 3):
                    for h in range(HPC):
                        if j < nj:
                            emit_sc(h, j)
                        if j >= 3:
                            emit_pv(h, j - 3)
                # normalize rows 0..63 by denominator row 64 (DRAM-bounce
                # broadcast: engines cannot replicate a row across
                # partitions)
                for h in range(HPC):
                    hh = h * Dh
                    rec = normtmp.tile([1, 512], F32, tag="rec")
                    nc.vector.reciprocal(rec[:], cps[h][64:65, :])
                    scr = drampool.tile([1, 512], F32, tag="scr")
                    nc.sync.dma_start(out=scr[:], in_=rec[:])
                    bc = normtmp.tile([64, 512], F32, tag="bc")
                    nc.sync.dma_start(out=bc[:],
                                       in_=scr[:].partition_broadcast(64))
                    nc.vector.tensor_mul(ctxt[4 * b + i][hh:hh + Dh, :],
                                         cps[h][0:64, :], bc[:])

            def emit_c_chunk(n2):
                """Output projection (column-shard partial) for one
                512-token chunk."""
                for e in range(DIM // 128):
                    ps = psA.tile([128, 512], F32, tag="proj",
                                  name=f"op{e}_{n2}")
                    nc.tensor.matmul(ps[:], wo_s[:, e * 128:(e + 1) * 128],
                                     ctxt[n2][:, :], start=True, stop=True)
                    ev = outev.tile([128, 512], F32, tag="ev")
                    if e % 2 == 0:
                        nc.vector.tensor_copy(ev[:], ps[:])
                    else:
                        nc.scalar.activation(ev[:], ps[:], CPY)
                    nc.sync.dma_start(
                        out=outT[e * 128:(e + 1) * 128,
                                 n2 * 512:(n2 + 1) * 512],
                        in_=ev[:])

            for _rep in range(reps):
                # Interleaved emission: engines execute their streams in
                # program order, so phase overlap must be baked into the
                # order. b=1 projections interleave with b=0 attention;
                # output projection interleaves with b=1 attention.
                emit_late_consts()
                for i in range(4):
                    emit_a_chunk(i)
                    emit_b_pair(0, i)
                for i in range(4):
                    emit_a_chunk(4 + i)
                    emit_c_chunk(i)
                    if i > 0:
                        emit_c_chunk(3 + i)
                    emit_b_pair(1, i)
                emit_c_chunk(7)
    return nc


_NC_CACHE = {}


def _get_nc(use_pad_mask: bool, reps: int = 1):
    key = (use_pad_mask, reps)
    if key not in _NC_CACHE:
        _NC_CACHE[key] = _build_nc(use_pad_mask, reps)
    return _NC_CACHE[key]


def _host_inputs(x, attention_mask, qkv_w, qkv_b, out_w, use_pad_mask):
    """Build the 8 per-core input maps."""
    xT = np.ascontiguousarray(
        x.reshape(NT, DIM).T.astype(np.float32))            # [1024, 4096]
    C, S = _rope_tables()
    swapm = _swap_matrix()
    ident = np.eye(128, dtype=np.float32)
    masks = _tri_masks()
    perm = _deinterleave_perm()

    qkv_w = np.asarray(qkv_w, dtype=np.float32)
    qkv_b = np.asarray(qkv_b, dtype=np.float32)
    out_w = np.asarray(out_w, dtype=np.float32)

    in_maps = []
    for c in range(NCORES):
        heads = [HPC * c + h for h in range(HPC)]
        # q/k rows get the de-interleave permutation; q gets the 1/sqrt(Dh)
        qrows = np.concatenate([h * Dh + perm for h in heads])
        vrows = np.concatenate(
            [h * Dh + np.arange(Dh) for h in heads])
        wq_c = qkv_w[qrows, :] * SCALE                       # [128, 1024]
        wk_c = qkv_w[DIM + qrows, :]
        wv_c = qkv_w[2 * DIM + vrows, :]
        def pack_w(w_c):
            # SBUF layout [128, 1024]: row p, cols kc*128+m hold
            # W^T[kc*128+p, m] - one contiguous DMA per weight
            return np.ascontiguousarray(
                w_c.T.reshape(8, 128, RPC).transpose(1, 0, 2).reshape(
                    128, DIM))

        m = {
            "xT": xT,
            "wq": pack_w(wq_c),
            "wk": pack_w(wk_c),
            "wv": pack_w(wv_c),
            "bq": np.ascontiguousarray(
                (qkv_b[qrows] * SCALE).reshape(RPC, 1)),
            "bk": np.ascontiguousarray(qkv_b[DIM + qrows].reshape(RPC, 1)),
            "bv": np.ascontiguousarray(
                qkv_b[2 * DIM + vrows].reshape(RPC, 1)),
            "ropec": C,
            "ropes": S,
            "swapm": swapm,
            "ident": ident,
            "masks": masks,
            "wo": np.ascontiguousarray(
                out_w[:, c * RPC:(c + 1) * RPC].T),          # [128, 1024]
            "ones64": np.ones((1, Dh), dtype=np.float32),
        }
        if use_pad_mask:
            pad = np.asarray(attention_mask, dtype=np.float32)  # [B, T]
            m["padv"] = np.ascontiguousarray(
                pad.reshape(B, T // 128, 128).transpose(0, 2, 1))
        in_maps.append(m)
    return in_maps


def kernel(x, attention_mask, qkv_w, qkv_b, out_w, out_b):
    from concourse.bass_utils import run_bass_kernel_spmd

    use_pad_mask = not np.asarray(attention_mask).all()
    nc = _get_nc(use_pad_mask)
    in_maps = _host_inputs(x, attention_mask, qkv_w, qkv_b, out_w,
                           use_pad_mask)
    res = run_bass_kernel_spmd(nc, in_maps, list(range(NCORES)))
    acc = res.results[0]["outT"].astype(np.float32)
    for c in range(1, NCORES):
        acc = acc + res.results[c]["outT"]
    out = acc.T + np.asarray(out_b, dtype=np.float32)[None, :]
    return np.ascontiguousarray(out.reshape(B, T, DIM), dtype=np.float32)



# revision 28
# speedup vs baseline: 1.0618x; 1.0618x over previous
"""Trainium2 Bass kernel for a RoPE causal-attention layer.

Problem (hardcoded): B=2, T=2048, DIM=1024, H=16 heads, Dh=64, fp32 I/O.
  qkv = x @ qkv_w.T + qkv_b ; rope(q), rope(k) ; causal softmax(q k^T / 8) @ v
  out = ctx @ out_w.T + out_b

Sharding: tensor-parallel over heads - each of the 8 cores owns 2 heads
(qkv_w row-sharded, out_w column-sharded); per-core partial outputs are
summed on the host (outT partials ship as bf16).

v2 design (vs the fp32r v1 baseline at 225us):
  * QKV projection on the PE in fp8-e4m3 DoubleRow perf mode (two K=128
    tiles summed per instruction at 0.5 cycles/column) with 3-term error
    compensation: W_hi*x_hi + W_hi*x_lo + W_lo*x_hi (hi = e4m3, lo = e4m3
    of the residual).  Weights pre-scaled by 256 to escape e4m3 subnormals;
    the (1/256)*psum + bias descale is one fused Pool tensor_scalar that
    also converts to bf16.
  * All other matmuls (scores, PV, out-proj) in bf16; PSUM stays f32.
  * RoPE: rows are 16-interleaved (A0 B0 A1 B1 per head; A/B = halves of
    the de-interleaved pair layout) so the half-swap is a DVE
    stream_shuffle (lane +-16 inside each 32-partition group) instead of a
    PE permutation matmul.  cos/sin ship pre-permuted in bf16.
  * V transposes into the PV operand layout via DMA XBAR transposes
    (SBUF->SBUF bf16); a ones column per 128-token block yields softmax
    denominators through the PV matmul (row 64 of the context PSUM).
  * Scores are transposed [s,t]; j-blocks pair into [128,1024] PSUM tiles
    so exp runs 1024 wide; diagonal blocks get partial-width exps and a
    [128,128] triangle mask multiply.  PSUM: 2 banks proj/out-proj + 4
    banks score pairs (both heads in flight) + 2 banks ctx = 8.
  * Engine placement: Act = exp only; Pool = bias descale + part of the
    PSUM evictions; DVE = rope/masks/normalize/reciprocal + the rest.
  * DMA count is minimized (fixed ~600ns HWDGE + SEQ cost per transfer):
    one fp8 x-chunk load per 512 tokens, single concatenated weight/bias/
    rope-table loads, out-proj staged through [128,<=2048] bf16 tiles, and
    one DRAM-bounce denominator broadcast per (batch, query block).
  * Out-proj work is emitted as fine-grained filler quanta inside the
    batch-1 attention emission so the PE never idles while Act runs exp.
"""

import sys

if "/opt/trn_rl_repo" not in sys.path:
    sys.path.insert(0, "/opt/trn_rl_repo")

import numpy as np

import concourse.bass as bass
import concourse.tile as tile
from concourse import mybir
from concourse.vector_clock import ScopedClock, VectorClock

B, T, DIM = 2, 2048, 1024
H, Dh = 16, 64
NCORES = 8
HPC = H // NCORES          # heads per core
NT = B * T                 # 4096 tokens
RPC = HPC * Dh             # 128 q/k/v rows per core
NQC = NT // 512            # 8 token chunks
SCALE = Dh ** -0.5
WSCALE = 256.0             # fp8 weight pre-scale (escape e4m3 subnormals)

F32 = mybir.dt.float32
BF16 = mybir.dt.bfloat16
F8 = mybir.dt.float8e4

SWAP16 = [(i + 16) % 32 for i in range(32)]


def _patch_tile_drain():
    """This container's walrus build allows at most ONE semaphore wait per
    instruction (setupSyncWait rejects more).  Two fixes:
      1. Tile's end-of-kernel drain accumulates one wait per live
         semaphore - split into one drain per semaphore.
      2. Any scheduled instruction that received >1 sem waits in stage 1B
         gets its extra waits hoisted onto same-engine NoOps emitted just
         before it during lowering."""
    if getattr(tile.TileContext, "_drain_patched", False):
        return

    def patched(self, tick_clock, wait_clock):
        vec = list(tick_clock.global_clock)
        nz = [(i, t) for i, t in enumerate(vec) if t > 0] or [(0, 0)]
        for i, t in nz:
            cv = [0] * len(vec)
            cv[i] = t
            d = self.nc.sync.drain()
            wait_clock.add_sem_waits(d.ins, ScopedClock({None: VectorClock(cv)}))
        self.nc.all_engine_barrier()
        popped = self.nc._tile_sem_poison_stack.pop()
        assert popped is self._sem_poison
        self.nc.clear_and_free_semaphores(list(self.sems.allocated().values()))
        self.nc.all_engine_barrier()

    tile.TileContext._drain_and_barrier = patched

    orig_cal = tile.TileContext._commit_and_lower

    def patched_cal(self, inst, original_block, old_bb_map, bb_to_exit_bb):
        si = getattr(inst, "sync_info", None)
        eng = getattr(inst, "engine", None)
        if si is not None and si.on_wait and eng in self.nc.engines:
            waits = list(si.on_wait)
            keep = 1
            if len(waits) > keep:
                for w in waits[: len(waits) - keep]:
                    nop = self.nc.engines[eng].nop(nofuse=True)
                    nop.ins.sync_info = mybir.SyncInfo(on_wait=[w], on_update=[])
                inst.sync_info = mybir.SyncInfo(
                    on_wait=waits[len(waits) - keep:],
                    on_update=list(si.on_update or []),
                )
        return orig_cal(self, inst, original_block, old_bb_map, bb_to_exit_bb)

    tile.TileContext._commit_and_lower = patched_cal
    tile.TileContext._drain_patched = True


def _deint_pos():
    """dp[p]: de-interleaved-layout position feeding final row p (64-row
    head).  Final blocks of 16: [A0, B0, A1, B1] with A = deint 0..31
    (pair element 0), B = deint 32..63 (element 1).  RoPE partners land
    +-16 apart inside each 32-partition group -> stream_shuffle SWAP16."""
    dp = np.empty(64, dtype=np.int64)
    dp[0:16] = np.arange(0, 16)
    dp[16:32] = np.arange(32, 48)
    dp[32:48] = np.arange(16, 32)
    dp[48:64] = np.arange(48, 64)
    return dp


def _row_perm():
    """packed_q[p] = q[perm[p]] within one head's 64 rows."""
    deint = np.empty(64, dtype=np.int64)
    deint[:32] = 2 * np.arange(32)
    deint[32:] = 2 * np.arange(32) + 1
    return deint[_deint_pos()]


def _rope_tables():
    """C, S [128, T] f32 in the final row layout (2 heads stacked)."""
    half = 32
    inv_freq = 1.0 / 10000.0 ** (np.arange(half, dtype=np.float64) / half)
    pair_freq = inv_freq[(2 * np.arange(half)) % half]
    ang = np.outer(pair_freq, np.arange(T, dtype=np.float64))
    c32, s32 = np.cos(ang), np.sin(ang)
    Cd = np.concatenate([c32, c32], axis=0)
    Sd = np.concatenate([-s32, s32], axis=0)
    dp = _deint_pos()
    C64, S64 = Cd[dp], Sd[dp]
    C = np.concatenate([C64, C64], axis=0)
    S = np.concatenate([S64, S64], axis=0)
    return C, S


def _build_nc(use_pad_mask: bool):
    _patch_tile_drain()
    nc = bass.Bass("TRN2", target_bir_lowering=False, debug=False,
                   num_devices=NCORES)

    MUL = mybir.AluOpType.mult
    ADD = mybir.AluOpType.add
    DR = mybir.MatmulPerfMode.DoubleRow
    EXP = mybir.ActivationFunctionType.Exp
    IDN = mybir.ActivationFunctionType.Identity

    # x8: (p, j kpair, hl, s slot, t) flattened to [128, 16, NT]:
    # col block 4*j + 2*hl + s.
    x8 = nc.dram_tensor("x8", [128, 16, NT], F8, kind="ExternalInput")
    # wcat: 6 weight planes (wqh wql wkh wkl wvh wvl), each (j, s, row):
    # [128, 6, 4, 2, 128]
    wcat = nc.dram_tensor("wcat", [128, 6, 4, 2, RPC], F8,
                          kind="ExternalInput")
    bcat = nc.dram_tensor("bcat", [RPC, 3], F32, kind="ExternalInput")
    ropecs = nc.dram_tensor("ropecs", [RPC, 2 * T], BF16,
                            kind="ExternalInput")
    maskt = nc.dram_tensor("maskt", [128, 128], BF16, kind="ExternalInput")
    wo = nc.dram_tensor("wo", [RPC, DIM], BF16, kind="ExternalInput")
    if use_pad_mask:
        padv = nc.dram_tensor("padv", [B, 128, T // 128], F32,
                              kind="ExternalInput")
    outT = nc.dram_tensor("outT", [DIM, NT], BF16, kind="ExternalOutput")

    with tile.TileContext(nc) as tc:
        with (
            tc.tile_pool(name="consts", bufs=1) as consts,
            tc.tile_pool(name="persist", bufs=1) as persist,
            tc.tile_pool(name="xpool", bufs=2) as xpool,
            tc.tile_pool(name="qkvtmp", bufs=4) as qkvtmp,
            tc.tile_pool(name="ropetmp", bufs=4) as ropetmp,
            tc.tile_pool(name="exppool", bufs=4) as exppool,
            tc.tile_pool(name="normtmp", bufs=2) as normtmp,
            tc.tile_pool(name="outev", bufs=3) as outev,
            tc.tile_pool(name="drampool", bufs=2, space="DRAM") as drampool,
            tc.tile_pool(name="psQ", bufs=2, space="PSUM") as psQ,
            tc.tile_pool(name="psS", bufs=2, space="PSUM") as psS,
            tc.tile_pool(name="psC", bufs=2, space="PSUM") as psC,
        ):
            # ---- constants: wcat ahead of x0; the rest are emitted by
            # emit_late_consts() between the first two A chunks so the x0
            # transfer isn't pushed out on the serial DMA path ------------
            w_s = consts.tile([128, 6, 4, 2, RPC], F8, tag="wcat")

            def load_wplane(k):
                nc.sync.dma_start(out=w_s[:, k], in_=wcat[:, k])
            b_s = consts.tile([RPC, 3], F32, tag="bcat")
            cs_s = consts.tile([RPC, 2 * T], BF16, tag="ropecs")
            mask_s = consts.tile([128, 128], BF16, tag="maskt")
            ones64 = consts.tile([1, 64], BF16, tag="ones64")
            nc.vector.memset(ones64[:], 1.0)
            wo_s = consts.tile([RPC, DIM], BF16, tag="wo")
            if use_pad_mask:
                pad_s = consts.tile([128, B * (T // 128)], F32, tag="padv")

            def emit_late_consts():
                nc.sync.dma_start(out=cs_s[:], in_=ropecs[:])
                nc.sync.dma_start(out=b_s[:], in_=bcat[:])
                nc.sync.dma_start(out=mask_s[:], in_=maskt[:])
                if use_pad_mask:
                    for b in range(B):
                        nc.sync.dma_start(
                            out=pad_s[:, b * 16:(b + 1) * 16], in_=padv[b])
                # wo on the SWDGE ring; its consumer is far downstream.
                nc.gpsimd.dma_start(out=wo_s[:], in_=wo[:])

            ropec_s = cs_s[:, 0:T]
            ropes_s = cs_s[:, T:2 * T]
            W = {"wqh": 0, "wql": 1, "wkh": 2, "wkl": 3, "wvh": 4, "wvl": 5}

            # ---- persistent activations ----------------------------------
            qrot = [persist.tile([RPC, 512], BF16, tag=f"qrot{n}",
                                 name=f"qrot{n}") for n in range(NQC)]
            krot = [persist.tile([RPC, 512], BF16, tag=f"krot{n}",
                                 name=f"krot{n}") for n in range(NQC)]
            ctxt = [persist.tile([RPC, 512], BF16, tag=f"ctxt{n}",
                                 name=f"ctxt{n}") for n in range(NQC)]
            # vaug block layout (132 cols per 128-token block):
            #   [h0 ch0..63, one0, pad, h1 ch0..63, one1, pad]
            # The XBAR transpose lands a 128-token block in a contiguous
            # staging tile; two Pool copies split it into the two heads'
            # slots so each PV operand is a contiguous 65-col slice
            # [64ch + ones] with context rows at partition base 0 and the
            # denominator on row 64 (HW requires partition bases 0/32/64).
            vaug = {}
            for b in range(B):
                va = persist.tile([128, 16 * 132], BF16, tag=f"vaug{b}")
                nc.gpsimd.memset(va[:], 1.0)
                vaug[b] = va

            def rope(raw, dst, tloc):
                sw = ropetmp.tile([128, 512], BF16, tag="sw")
                nc.vector.stream_shuffle(sw[:], raw[:], SWAP16)
                nc.vector.tensor_mul(dst, raw[:],
                                     ropec_s[:, tloc:tloc + 512])
                sw2 = ropetmp.tile([128, 512], BF16, tag="sw2")
                nc.vector.tensor_mul(sw2[:], sw[:],
                                     ropes_s[:, tloc:tloc + 512])
                nc.vector.tensor_add(dst, dst, sw2[:])

            pending_vt = []

            def flush_vt():
                while pending_vt:
                    pending_vt.pop(0)()

            xt_tiles = {}

            def load_x(n):
                xt = xpool.tile([128, 16, 512], F8, tag="xt", name=f"xt{n}")
                nc.sync.dma_start(out=xt[:],
                                  in_=x8[:, :, n * 512:(n + 1) * 512])
                xt_tiles[n] = xt

            def emit_a_chunk(n):
                """QKV fp8 projection + RoPE + V DMA-transpose for one
                512-token chunk.  x for chunk n+1 is prefetched first;
                deferred work from earlier chunks (denominator broadcast,
                V transposes, normalize muls) is flushed at points where
                its waits are already satisfied."""
                if n + 1 < NQC:
                    load_x(n + 1)
                flush_vt()
                t0 = n * 512
                tloc = t0 % T
                b = t0 // T
                xt = xt_tiles.pop(n)

                def xs(j, hl):
                    return xt[:, 4 * j + 2 * hl:4 * j + 2 * hl + 2, :]

                def project(hi, lo, bi, dst, critical):
                    ps = psQ.tile([128, 512], F32, tag="proj")
                    terms = [(hi, 0), (hi, 1), (lo, 0)]
                    for ti, (w, hl) in enumerate(terms):
                        for j in range(4):
                            nc.tensor.matmul(
                                ps[:], w_s[:, W[w], j], xs(j, hl),
                                start=(ti == 0 and j == 0),
                                stop=(ti == 2 and j == 3),
                                perf_mode=DR)
                    # (1/256)*psum + bias, f32 PSUM -> bf16 SBUF.  GPSIMD
                    # has no PSUM access, so this is Act (q/k: feeds RoPE,
                    # short queue) or DVE (v: consumed late).
                    if critical:
                        nc.scalar.activation(dst[:], ps[:], IDN,
                                             bias=b_s[:, bi:bi + 1],
                                             scale=1.0 / WSCALE)
                    else:
                        nc.vector.tensor_scalar(
                            out=dst[:], in0=ps[:], scalar1=1.0 / WSCALE,
                            scalar2=b_s[:, bi:bi + 1], op0=MUL, op1=ADD)

                qraw = qkvtmp.tile([128, 512], BF16, tag="qraw")
                project("wqh", "wql", 0, qraw, critical=True)
                kraw = qkvtmp.tile([128, 512], BF16, tag="kraw")
                project("wkh", "wkl", 1, kraw, critical=True)
                rope(qraw, qrot[n][:, :], tloc)
                vraw = qkvtmp.tile([128, 512], BF16, tag="vraw",
                                   name=f"vraw{n}")
                project("wvh", "wvl", 2, vraw, critical=False)
                rope(kraw, krot[n][:, :], tloc)
                va = vaug[b]

                def vtrans():
                    for m in range(4):
                        blk = (tloc + m * 128) // 128
                        vs = qkvtmp.tile([128, 128], BF16, tag="vstage",
                                         name=f"vs{n}_{m}")
                        nc.sync.dma_start_transpose(
                            vs[:], vraw[:, m * 128:(m + 1) * 128])
                        for h in range(HPC):
                            nc.gpsimd.tensor_copy(
                                va[:, blk * 132 + 66 * h:
                                   blk * 132 + 66 * h + 64],
                                vs[:, 64 * h:64 * (h + 1)])
                pending_vt.append(vtrans)

            def emit_b_pair(b, i, filler=None):
                """Flash attention for one (batch, 512-query block), both
                heads software-pipelined; j-blocks pair into [128,1024]
                PSUM tiles per head, heads alternating, so exp runs 1024
                wide and each exp has a full other-head cycle to
                complete."""
                nj = 4 * (i + 1)
                P = nj // 2
                n = 4 * b + i
                cps = {}
                for h in range(HPC):
                    cps[h] = psC.tile([65, 512], F32, tag="cps",
                                      name=f"cps{b}{h}{i}")
                va = vaug[b]
                grp = {}
                exs = {}

                def j_c0(j):
                    kdiag = j - (nj - 4)
                    return 128 * kdiag if kdiag > 0 else 0

                def emit_sc(h, p):
                    hh = h * Dh
                    g = psS.tile([128, 1024], F32, tag="grp",
                                 name=f"g{b}{i}{h}{p}")
                    for u in range(2):
                        j = 2 * p + u
                        kchunk = (b * T + j * 128) // 512
                        koff = (j * 128) % 512
                        # u=1 computes full width even on diagonal blocks
                        # so exp can run as one contiguous instruction;
                        # the acausal columns are never read by PV.
                        c0 = j_c0(j) if u == 0 else 0
                        nc.tensor.matmul(
                            g[:, 512 * u + c0:512 * (u + 1)],
                            krot[kchunk][hh:hh + Dh, koff:koff + 128],
                            qrot[n][hh:hh + Dh, c0:512],
                            start=True, stop=True)
                    grp[(h, p)] = g

                def emit_exp(h, p):
                    g = grp.pop((h, p))
                    ex = exppool.tile([128, 1024], BF16, tag="ex")
                    j0, j1 = 2 * p, 2 * p + 1
                    c0 = j_c0(j0)
                    nc.scalar.activation(ex[:, c0:1024], g[:, c0:1024], EXP)
                    for u, j in enumerate((j0, j1)):
                        kdiag = j - (nj - 4)
                        if kdiag >= 0:
                            dc = 512 * u + 128 * kdiag
                            nc.gpsimd.tensor_mul(
                                ex[:, dc:dc + 128], ex[:, dc:dc + 128],
                                mask_s[:])
                        if use_pad_mask:
                            c0 = j_c0(j)
                            nc.vector.tensor_scalar_mul(
                                ex[:, 512 * u + c0:512 * (u + 1)],
                                ex[:, 512 * u + c0:512 * (u + 1)],
                                pad_s[:, b * 16 + j:b * 16 + j + 1])
                    exs[(h, p)] = ex

                def emit_pv(h, p):
                    ex = exs.pop((h, p))
                    for u in range(2):
                        j = 2 * p + u
                        c0 = j_c0(j)
                        nc.tensor.matmul(
                            cps[h][:, c0:512],
                            va[:, j * 132 + 66 * h:j * 132 + 66 * h + 65],
                            ex[:, 512 * u + c0:512 * (u + 1)],
                            start=(j == 0), stop=(j == nj - 1))

                def fill(k):
                    if filler is not None:
                        for _ in range(k):
                            if not filler():
                                return

                emit_sc(0, 0)
                emit_sc(1, 0)
                for p in range(P):
                    emit_exp(0, p)
                    if p + 1 < P:
                        emit_sc(0, p + 1)
                    fill(1)
                    emit_pv(0, p)
                    emit_exp(1, p)
                    if p + 1 < P:
                        emit_sc(1, p + 1)
                    fill(1)
                    emit_pv(1, p)

                # normalize rows 0..63 by denominator row 64.  Engines
                # cannot replicate a row across partitions, but the PE can:
                # bcast[64h:64h+64, :] = ones64^T @ recip_row_h (K=1 matmul)
                # into a score-pool PSUM tile.  No DMA round-trip, so ctxt
                # is ready ~2us after the last PV and nothing downstream
                # ever waits on a multi-us broadcast chain.
                recs = [normtmp.tile([1, 512], BF16, tag=f"rec{h}",
                                     name=f"rec{b}{i}{h}")
                        for h in range(HPC)]
                with nc.allow_low_precision(reason="bf16 softmax denom"):
                    for h in range(HPC):
                        nc.vector.reciprocal(recs[h][:], cps[h][64:65, :])
                fill(2)
                bcp = psQ.tile([128, 512], F32, tag="proj",
                               name=f"bcp{b}{i}")
                for h in range(HPC):
                    nc.tensor.matmul(bcp[64 * h:64 * (h + 1), :],
                                     ones64[:], recs[h][:],
                                     start=True, stop=True)
                # engines cannot read two PSUM operands in one op: stage
                # the broadcast rows through SBUF
                bcs = normtmp.tile([128, 512], F32, tag="bcs",
                                   name=f"bcs{b}{i}")
                nc.vector.tensor_copy(bcs[:], bcp[:])
                for h in range(HPC):
                    hh = h * Dh
                    nc.vector.tensor_mul(ctxt[n][hh:hh + Dh, :],
                                         cps[h][0:64, :],
                                         bcs[64 * h:64 * (h + 1), :])

            # ---- out-projection as filler quanta --------------------------
            # One (e, chunk) matmul + eviction per quantum; the outT DMA
            # is issued one quantum LATE on the Activation HWDGE ring so
            # its eviction wait is already satisfied at issue time.
            ev_alt = [0]
            cqueue = []
            cstate = {"dma": None}

            def c_quantum():
                st = cstate
                if st["dma"] is not None:
                    st["dma"]()
                    st["dma"] = None
                if not cqueue:
                    return False
                e, n2 = cqueue.pop(0)
                ps = psQ.tile([128, 512], F32, tag="proj",
                              name=f"op{e}_{n2}")
                nc.tensor.matmul(ps[:], wo_s[:, e * 128:(e + 1) * 128],
                                 ctxt[n2][:, :], start=True, stop=True)
                ev = outev.tile([128, 512], BF16, tag="ev",
                                name=f"ev{e}_{n2}")
                if ev_alt[0] % 4 == 3:
                    nc.scalar.activation(ev[:], ps[:], IDN)
                else:
                    nc.vector.tensor_copy(ev[:], ps[:])
                ev_alt[0] += 1

                def dma(ev=ev, e=e, n2=n2):
                    nc.sync.dma_start(
                        out=outT[e * 128:(e + 1) * 128,
                                 n2 * 512:(n2 + 1) * 512],
                        in_=ev[:])
                st["dma"] = dma
                return True

            def drain_c(k):
                for _ in range(k):
                    if not c_quantum():
                        return

            # ---- schedule -------------------------------------------------
            load_wplane(0)
            load_wplane(1)
            load_x(0)
            for _k in range(2, 6):
                load_wplane(_k)
            emit_late_consts()
            emit_a_chunk(0)
            emit_a_chunk(1)
            flush_vt()
            emit_b_pair(0, 0)
            cqueue.extend((e, 0) for e in range(8))
            emit_a_chunk(2)
            drain_c(2)
            emit_b_pair(0, 1, filler=c_quantum)
            cqueue.extend((e, 1) for e in range(8))
            emit_a_chunk(3)
            drain_c(4)
            emit_b_pair(0, 2, filler=c_quantum)
            cqueue.extend((e, 2) for e in range(8))
            emit_a_chunk(4)
            drain_c(4)
            emit_b_pair(0, 3, filler=c_quantum)
            cqueue.extend((e, 3) for e in range(8))
            emit_a_chunk(5)
            drain_c(4)
            emit_b_pair(1, 0, filler=c_quantum)
            cqueue.extend((e, 4) for e in range(8))
            emit_a_chunk(6)
            drain_c(4)
            emit_b_pair(1, 1, filler=c_quantum)
            cqueue.extend((e, 5) for e in range(8))
            emit_a_chunk(7)
            drain_c(4)
            emit_b_pair(1, 2, filler=c_quantum)
            cqueue.extend((e, 6) for e in range(8))
            flush_vt()
            drain_c(4)
            emit_b_pair(1, 3, filler=c_quantum)
            cqueue.extend((e, 7) for e in range(8))
            drain_c(10 ** 6)
            if cstate["dma"] is not None:
                cstate["dma"]()
                cstate["dma"] = None
    return nc


_NC_CACHE = {}


def _get_nc(use_pad_mask: bool):
    if use_pad_mask not in _NC_CACHE:
        _NC_CACHE[use_pad_mask] = _build_nc(use_pad_mask)
    return _NC_CACHE[use_pad_mask]


def _host_inputs(x, attention_mask, qkv_w, qkv_b, out_w, use_pad_mask):
    import ml_dtypes
    E4 = ml_dtypes.float8_e4m3fn
    BF = ml_dtypes.bfloat16

    x = np.asarray(x, dtype=np.float32)
    qkv_w = np.asarray(qkv_w, dtype=np.float32)
    qkv_b = np.asarray(qkv_b, dtype=np.float32)
    out_w = np.asarray(out_w, dtype=np.float32)

    # x -> [128, (j, hl, s), NT] fp8 with hi/lo planes
    xT = x.reshape(NT, DIM).T                                # [1024, NT]
    xk = xT.reshape(4, 2, 128, NT).transpose(2, 0, 1, 3)     # [128,4,2,NT]
    xh = xk.astype(E4)
    xl = (xk - xh.astype(np.float32)).astype(E4)
    x8 = np.empty((128, 4, 2, 2, NT), dtype=E4)
    x8[:, :, 0, :, :] = xh
    x8[:, :, 1, :, :] = xl
    x8 = np.ascontiguousarray(x8.reshape(128, 16, NT))

    C, S = _rope_tables()
    ropecs = np.ascontiguousarray(
        np.concatenate([C, S], axis=1).astype(BF))
    perm = _row_perm()
    tri = (np.arange(128)[:, None] <= np.arange(128)[None, :])
    maskt = np.ascontiguousarray(tri.astype(BF))

    def pack_w(w_c):
        """[128 rows, 1024 K] -> hi/lo fp8 [128p, 4 kpair, 2 slot, 128 row]:
        element (p, j, s, m) = scaled W[m, 256j + 128s + p]."""
        ws = w_c * WSCALE
        wt = ws.T.reshape(4, 2, 128, RPC).transpose(2, 0, 1, 3)
        hi = wt.astype(E4)
        lo = (wt - hi.astype(np.float32)).astype(E4)
        return hi, lo

    in_maps = []
    for c in range(NCORES):
        heads = [HPC * c + h for h in range(HPC)]
        qrows = np.concatenate([h * Dh + perm for h in heads])
        vrows = np.concatenate([h * Dh + np.arange(Dh) for h in heads])
        wq_hi, wq_lo = pack_w(qkv_w[qrows, :] * SCALE)
        wk_hi, wk_lo = pack_w(qkv_w[DIM + qrows, :])
        wv_hi, wv_lo = pack_w(qkv_w[2 * DIM + vrows, :])
        wcat = np.ascontiguousarray(np.stack(
            [wq_hi, wq_lo, wk_hi, wk_lo, wv_hi, wv_lo], axis=1))
        bcat = np.ascontiguousarray(np.stack(
            [qkv_b[qrows] * SCALE, qkv_b[DIM + qrows],
             qkv_b[2 * DIM + vrows]], axis=1).astype(np.float32))
        m = {
            "x8": x8,
            "wcat": wcat,
            "bcat": bcat,
            "ropecs": ropecs,
            "maskt": maskt,
            "wo": np.ascontiguousarray(
                out_w[:, c * RPC:(c + 1) * RPC].T.astype(BF)),
        }
        if use_pad_mask:
            pad = np.asarray(attention_mask, dtype=np.float32)
            m["padv"] = np.ascontiguousarray(
                pad.reshape(B, T // 128, 128).transpose(0, 2, 1))
        in_maps.append(m)
    return in_maps


def kernel(x, attention_mask, qkv_w, qkv_b, out_w, out_b):
    from concourse.bass_utils import run_bass_kernel_spmd

    use_pad_mask = not np.asarray(attention_mask).all()
    nc = _get_nc(use_pad_mask)
    in_maps = _host_inputs(x, attention_mask, qkv_w, qkv_b, out_w,
                           use_pad_mask)
    res = run_bass_kernel_spmd(nc, in_maps, list(range(NCORES)))
    acc = res.results[0]["outT"].astype(np.float32)
    for c in range(1, NCORES):
        acc = acc + res.results[c]["outT"].astype(np.float32)
    out = acc.T + np.asarray(out_b, dtype=np.float32)[None, :]
    return np.ascontiguousarray(out.reshape(B, T, DIM), dtype=np.float32)


# revision 38
# speedup vs baseline: 1.2269x; 1.1555x over previous
"""Trainium2 Bass kernel for a RoPE causal-attention layer.

Problem (hardcoded): B=2, T=2048, DIM=1024, H=16 heads, Dh=64, fp32 I/O.
  qkv = x @ qkv_w.T + qkv_b ; rope(q), rope(k) ; causal softmax(q k^T / 8) @ v
  out = ctx @ out_w.T + out_b

Sharding: tensor-parallel over heads - each of the 8 cores owns 2 heads
(qkv_w row-sharded, out_w column-sharded); per-core bf16 partial outputs
are summed on the host.  HW exec (cost-model timeline): 184.9us vs the
225.5us fp32r v1 baseline; measured rel err 4.3e-3 vs the 2e-2 gate.

Design:
  * QKV projection on the PE in fp8-e4m3 DoubleRow perf mode (two K=128
    tiles summed per instruction at 0.5 cycles/column) with 3-term error
    compensation: W_hi*x_hi + W_hi*x_lo + W_lo*x_hi (hi = e4m3, lo = e4m3
    of the residual).  Weights pre-scaled by 256 to escape e4m3
    subnormals; the (1/256)*psum + bias descale fuses into the eviction.
  * All other matmuls (scores, PV, out-proj) in bf16; PSUM stays f32.
  * RoPE: rows are 16-interleaved (A0 B0 A1 B1 per head; A/B = halves of
    the de-interleaved pair layout) so the half-swap is a DVE
    stream_shuffle (lane +-16 inside each 32-partition group) instead of
    a PE permutation matmul.  cos/sin ship pre-permuted in bf16.
  * V transposes via DMA XBAR (SBUF->SBUF bf16) into a staging tile; two
    Pool copies split each 128-token block into the per-head PV operand
    layout [64ch + ones] (blocks of 132 cols), giving contiguous 65-col
    matmul operands and softmax denominators on PSUM row 64 for free.
  * Scores are transposed [s,t]; per (head, j-pair) they land in a
    [128,1024] PSUM tile so exp runs 1024 wide (diagonal pairs compute
    the second block full-width so exp stays one instruction; acausal
    columns are never read by PV).  Triangle masks multiply on Pool.
  * Softmax normalization without any DMA round trip: DVE reciprocal of
    the denominator rows (bf16), then a K=1 ones-column PE matmul
    broadcasts each reciprocal row across 64 partitions into PSUM; one
    DVE copy stages it to SBUF and two DVE muls write normalized bf16
    context.  Context is ready ~2us after the last PV, so out-projection
    filler work never blocks the in-order PE queue.
  * Engine placement (GPSIMD cannot touch PSUM): Act = exps + q/k/v bias
    descales + boundary evictions; DVE = shuffles, rope cos-mul/add,
    reciprocal, normalize, in-block evictions; Pool = rope sin-mul,
    triangle masks, V-split copies, memsets.
  * Out-projection is emitted as (e, chunk) filler quanta inside the
    attention emission and at block boundaries, with the outT DMA issued
    one quantum late so its eviction wait is satisfied at issue time.
  * DMA count minimized (fixed ~600ns HWDGE + SEQ cost per transfer):
    one fp8 x-chunk load per 512 tokens (prefetched a chunk ahead), one
    concatenated weight tensor (6 plane DMAs, just-in-time at startup),
    batched rope tables, deferred V-transposes.
"""

import sys

if "/opt/trn_rl_repo" not in sys.path:
    sys.path.insert(0, "/opt/trn_rl_repo")

import numpy as np

import concourse.bass as bass
import concourse.tile as tile
from concourse import mybir
from concourse.vector_clock import ScopedClock, VectorClock

B, T, DIM = 2, 2048, 1024
H, Dh = 16, 64
NCORES = 8
HPC = H // NCORES          # heads per core
NT = B * T                 # 4096 tokens
RPC = HPC * Dh             # 128 q/k/v rows per core
NQC = NT // 512            # 8 token chunks
SCALE = Dh ** -0.5
WSCALE = 256.0             # fp8 weight pre-scale (escape e4m3 subnormals)

F32 = mybir.dt.float32
BF16 = mybir.dt.bfloat16
F8 = mybir.dt.float8e4

SWAP16 = [(i + 16) % 32 for i in range(32)]


def _patch_tile_drain():
    """This container's walrus build allows at most ONE semaphore wait per
    instruction (setupSyncWait rejects more).  Two fixes:
      1. Tile's end-of-kernel drain accumulates one wait per live
         semaphore - split into one drain per semaphore.
      2. Any scheduled instruction that received >1 sem waits in stage 1B
         gets its extra waits hoisted onto same-engine NoOps emitted just
         before it during lowering."""
    if getattr(tile.TileContext, "_drain_patched", False):
        return

    def patched(self, tick_clock, wait_clock):
        vec = list(tick_clock.global_clock)
        nz = [(i, t) for i, t in enumerate(vec) if t > 0] or [(0, 0)]
        for i, t in nz:
            cv = [0] * len(vec)
            cv[i] = t
            d = self.nc.sync.drain()
            wait_clock.add_sem_waits(d.ins, ScopedClock({None: VectorClock(cv)}))
        self.nc.all_engine_barrier()
        popped = self.nc._tile_sem_poison_stack.pop()
        assert popped is self._sem_poison
        self.nc.clear_and_free_semaphores(list(self.sems.allocated().values()))
        self.nc.all_engine_barrier()

    tile.TileContext._drain_and_barrier = patched

    orig_cal = tile.TileContext._commit_and_lower

    def patched_cal(self, inst, original_block, old_bb_map, bb_to_exit_bb):
        si = getattr(inst, "sync_info", None)
        eng = getattr(inst, "engine", None)
        if si is not None and si.on_wait and eng in self.nc.engines:
            waits = list(si.on_wait)
            keep = 1
            if len(waits) > keep:
                for w in waits[: len(waits) - keep]:
                    nop = self.nc.engines[eng].nop(nofuse=True)
                    nop.ins.sync_info = mybir.SyncInfo(on_wait=[w], on_update=[])
                inst.sync_info = mybir.SyncInfo(
                    on_wait=waits[len(waits) - keep:],
                    on_update=list(si.on_update or []),
                )
        return orig_cal(self, inst, original_block, old_bb_map, bb_to_exit_bb)

    tile.TileContext._commit_and_lower = patched_cal
    tile.TileContext._drain_patched = True


def _deint_pos():
    """dp[p]: de-interleaved-layout position feeding final row p (64-row
    head).  Final blocks of 16: [A0, B0, A1, B1] with A = deint 0..31
    (pair element 0), B = deint 32..63 (element 1).  RoPE partners land
    +-16 apart inside each 32-partition group -> stream_shuffle SWAP16."""
    dp = np.empty(64, dtype=np.int64)
    dp[0:16] = np.arange(0, 16)
    dp[16:32] = np.arange(32, 48)
    dp[32:48] = np.arange(16, 32)
    dp[48:64] = np.arange(48, 64)
    return dp


def _row_perm():
    """packed_q[p] = q[perm[p]] within one head's 64 rows."""
    deint = np.empty(64, dtype=np.int64)
    deint[:32] = 2 * np.arange(32)
    deint[32:] = 2 * np.arange(32) + 1
    return deint[_deint_pos()]


def _rope_tables():
    """C, S [128, T] f32 in the final row layout (2 heads stacked)."""
    half = 32
    inv_freq = 1.0 / 10000.0 ** (np.arange(half, dtype=np.float64) / half)
    pair_freq = inv_freq[(2 * np.arange(half)) % half]
    ang = np.outer(pair_freq, np.arange(T, dtype=np.float64))
    c32, s32 = np.cos(ang), np.sin(ang)
    Cd = np.concatenate([c32, c32], axis=0)
    Sd = np.concatenate([-s32, s32], axis=0)
    dp = _deint_pos()
    C64, S64 = Cd[dp], Sd[dp]
    C = np.concatenate([C64, C64], axis=0)
    S = np.concatenate([S64, S64], axis=0)
    return C, S


def _build_nc(use_pad_mask: bool):
    _patch_tile_drain()
    nc = bass.Bass("TRN2", target_bir_lowering=False, debug=False,
                   num_devices=NCORES)

    MUL = mybir.AluOpType.mult
    ADD = mybir.AluOpType.add
    DR = mybir.MatmulPerfMode.DoubleRow
    EXP = mybir.ActivationFunctionType.Exp
    IDN = mybir.ActivationFunctionType.Identity

    # x8: (p, j kpair, hl, s slot, t) flattened to [128, 16, NT]:
    # col block 4*j + 2*hl + s.
    x8 = nc.dram_tensor("x8", [128, 16, NT], F8, kind="ExternalInput")
    # wcat: 6 weight planes (wqh wql wkh wkl wvh wvl), each (j, s, row):
    # [128, 6, 4, 2, 128]
    wcat = nc.dram_tensor("wcat", [128, 6, 4, 2, RPC], F8,
                          kind="ExternalInput")
    bcat = nc.dram_tensor("bcat", [RPC, 3], F32, kind="ExternalInput")
    ropecs = nc.dram_tensor("ropecs", [RPC, 2 * T], BF16,
                            kind="ExternalInput")
    maskt = nc.dram_tensor("maskt", [128, 128], BF16, kind="ExternalInput")
    wo = nc.dram_tensor("wo", [RPC, DIM], BF16, kind="ExternalInput")
    if use_pad_mask:
        padv = nc.dram_tensor("padv", [B, 128, T // 128], F32,
                              kind="ExternalInput")
    outT = nc.dram_tensor("outT", [DIM, NT], BF16, kind="ExternalOutput")

    with tile.TileContext(nc) as tc:
        with (
            tc.tile_pool(name="consts", bufs=1) as consts,
            tc.tile_pool(name="persist", bufs=1) as persist,
            tc.tile_pool(name="xpool", bufs=2) as xpool,
            tc.tile_pool(name="qkvtmp", bufs=4) as qkvtmp,
            tc.tile_pool(name="ropetmp", bufs=4) as ropetmp,
            tc.tile_pool(name="exppool", bufs=4) as exppool,
            tc.tile_pool(name="normtmp", bufs=2) as normtmp,
            tc.tile_pool(name="outev", bufs=3) as outev,
            tc.tile_pool(name="drampool", bufs=2, space="DRAM") as drampool,
            tc.tile_pool(name="psQ", bufs=2, space="PSUM") as psQ,
            tc.tile_pool(name="psS", bufs=2, space="PSUM") as psS,
            tc.tile_pool(name="psC", bufs=2, space="PSUM") as psC,
        ):
            # ---- constants: wcat ahead of x0; the rest are emitted by
            # emit_late_consts() between the first two A chunks so the x0
            # transfer isn't pushed out on the serial DMA path ------------
            w_s = consts.tile([128, 6, 4, 2, RPC], F8, tag="wcat")

            def load_wplane(k):
                nc.sync.dma_start(out=w_s[:, k], in_=wcat[:, k])
            b_s = consts.tile([RPC, 3], F32, tag="bcat")
            cs_s = consts.tile([RPC, 2 * T], BF16, tag="ropecs")
            mask_s = consts.tile([128, 128], BF16, tag="maskt")
            ones64 = consts.tile([1, 64], BF16, tag="ones64")
            nc.vector.memset(ones64[:], 1.0)
            wo_s = consts.tile([RPC, DIM], BF16, tag="wo")
            if use_pad_mask:
                pad_s = consts.tile([128, B * (T // 128)], F32, tag="padv")

            def emit_late_consts():
                nc.sync.dma_start(out=cs_s[:], in_=ropecs[:])
                nc.sync.dma_start(out=b_s[:], in_=bcat[:])
                nc.sync.dma_start(out=mask_s[:], in_=maskt[:])
                if use_pad_mask:
                    for b in range(B):
                        nc.sync.dma_start(
                            out=pad_s[:, b * 16:(b + 1) * 16], in_=padv[b])
                # wo on the SWDGE ring; its consumer is far downstream.
                nc.gpsimd.dma_start(out=wo_s[:], in_=wo[:])

            ropec_s = cs_s[:, 0:T]
            ropes_s = cs_s[:, T:2 * T]
            W = {"wqh": 0, "wql": 1, "wkh": 2, "wkl": 3, "wvh": 4, "wvl": 5}

            # ---- persistent activations ----------------------------------
            qrot = [persist.tile([RPC, 512], BF16, tag=f"qrot{n}",
                                 name=f"qrot{n}") for n in range(NQC)]
            krot = [persist.tile([RPC, 512], BF16, tag=f"krot{n}",
                                 name=f"krot{n}") for n in range(NQC)]
            ctxt = [persist.tile([RPC, 512], BF16, tag=f"ctxt{n}",
                                 name=f"ctxt{n}") for n in range(NQC)]
            # vaug block layout (132 cols per 128-token block):
            #   [h0 ch0..63, one0, pad, h1 ch0..63, one1, pad]
            # The XBAR transpose lands a 128-token block in a contiguous
            # staging tile; two Pool copies split it into the two heads'
            # slots so each PV operand is a contiguous 65-col slice
            # [64ch + ones] with context rows at partition base 0 and the
            # denominator on row 64 (HW requires partition bases 0/32/64).
            vaug = {}
            for b in range(B):
                va = persist.tile([128, 16 * 132], BF16, tag=f"vaug{b}")
                nc.gpsimd.memset(va[:], 1.0)
                vaug[b] = va

            def rope(raw, dst, tloc):
                sw = ropetmp.tile([128, 512], BF16, tag="sw")
                nc.vector.stream_shuffle(sw[:], raw[:], SWAP16)
                nc.vector.tensor_mul(dst, raw[:],
                                     ropec_s[:, tloc:tloc + 512])
                sw2 = ropetmp.tile([128, 512], BF16, tag="sw2")
                nc.gpsimd.tensor_mul(sw2[:], sw[:],
                                     ropes_s[:, tloc:tloc + 512])
                nc.vector.tensor_add(dst, dst, sw2[:])

            pending_vt = []

            def flush_vt():
                while pending_vt:
                    pending_vt.pop(0)()

            xt_tiles = {}

            def load_x(n):
                xt = xpool.tile([128, 16, 512], F8, tag="xt", name=f"xt{n}")
                nc.sync.dma_start(out=xt[:],
                                  in_=x8[:, :, n * 512:(n + 1) * 512])
                xt_tiles[n] = xt

            def emit_a_chunk(n):
                """QKV fp8 projection + RoPE + V DMA-transpose for one
                512-token chunk.  x for chunk n+1 is prefetched first;
                deferred work from earlier chunks (denominator broadcast,
                V transposes, normalize muls) is flushed at points where
                its waits are already satisfied."""
                if n + 1 < NQC:
                    load_x(n + 1)
                flush_vt()
                t0 = n * 512
                tloc = t0 % T
                b = t0 // T
                xt = xt_tiles.pop(n)

                def xs(j, hl):
                    return xt[:, 4 * j + 2 * hl:4 * j + 2 * hl + 2, :]

                def project(hi, lo, bi, dst, critical):
                    ps = psQ.tile([128, 512], F32, tag="proj")
                    terms = [(hi, 0), (hi, 1), (lo, 0)]
                    for ti, (w, hl) in enumerate(terms):
                        for j in range(4):
                            nc.tensor.matmul(
                                ps[:], w_s[:, W[w], j], xs(j, hl),
                                start=(ti == 0 and j == 0),
                                stop=(ti == 2 and j == 3),
                                perf_mode=DR)
                    # (1/256)*psum + bias, f32 PSUM -> bf16 SBUF.  GPSIMD
                    # has no PSUM access, so this is Act (q/k: feeds RoPE,
                    # short queue) or DVE (v: consumed late).
                    if critical:
                        nc.scalar.activation(dst[:], ps[:], IDN,
                                             bias=b_s[:, bi:bi + 1],
                                             scale=1.0 / WSCALE)
                    else:
                        nc.vector.tensor_scalar(
                            out=dst[:], in0=ps[:], scalar1=1.0 / WSCALE,
                            scalar2=b_s[:, bi:bi + 1], op0=MUL, op1=ADD)

                qraw = qkvtmp.tile([128, 512], BF16, tag="qraw")
                project("wqh", "wql", 0, qraw, critical=True)
                kraw = qkvtmp.tile([128, 512], BF16, tag="kraw")
                project("wkh", "wkl", 1, kraw, critical=True)
                rope(qraw, qrot[n][:, :], tloc)
                vraw = qkvtmp.tile([128, 512], BF16, tag="vraw",
                                   name=f"vraw{n}")
                project("wvh", "wvl", 2, vraw, critical=False)
                rope(kraw, krot[n][:, :], tloc)
                va = vaug[b]

                def vtrans():
                    for m in range(4):
                        blk = (tloc + m * 128) // 128
                        vs = qkvtmp.tile([128, 128], BF16, tag="vstage",
                                         name=f"vs{n}_{m}")
                        nc.sync.dma_start_transpose(
                            vs[:], vraw[:, m * 128:(m + 1) * 128])
                        for h in range(HPC):
                            nc.gpsimd.tensor_copy(
                                va[:, blk * 132 + 66 * h:
                                   blk * 132 + 66 * h + 64],
                                vs[:, 64 * h:64 * (h + 1)])
                pending_vt.append(vtrans)

            def emit_b_pair(b, i, filler=None):
                """Flash attention for one (batch, 512-query block), both
                heads software-pipelined; j-blocks pair into [128,1024]
                PSUM tiles per head, heads alternating, so exp runs 1024
                wide and each exp has a full other-head cycle to
                complete."""
                nj = 4 * (i + 1)
                P = nj // 2
                n = 4 * b + i
                cps = {}
                for h in range(HPC):
                    cps[h] = psC.tile([65, 512], F32, tag="cps",
                                      name=f"cps{b}{h}{i}")
                va = vaug[b]
                grp = {}
                exs = {}

                def j_c0(j):
                    kdiag = j - (nj - 4)
                    return 128 * kdiag if kdiag > 0 else 0

                def emit_sc(h, p):
                    hh = h * Dh
                    g = psS.tile([128, 1024], F32, tag="grp",
                                 name=f"g{b}{i}{h}{p}")
                    for u in range(2):
                        j = 2 * p + u
                        kchunk = (b * T + j * 128) // 512
                        koff = (j * 128) % 512
                        # u=1 computes full width even on diagonal blocks
                        # so exp can run as one contiguous instruction;
                        # the acausal columns are never read by PV.
                        c0 = j_c0(j) if u == 0 else 0
                        nc.tensor.matmul(
                            g[:, 512 * u + c0:512 * (u + 1)],
                            krot[kchunk][hh:hh + Dh, koff:koff + 128],
                            qrot[n][hh:hh + Dh, c0:512],
                            start=True, stop=True)
                    grp[(h, p)] = g

                def emit_exp(h, p):
                    g = grp.pop((h, p))
                    ex = exppool.tile([128, 1024], BF16, tag="ex")
                    j0, j1 = 2 * p, 2 * p + 1
                    c0 = j_c0(j0)
                    nc.scalar.activation(ex[:, c0:1024], g[:, c0:1024], EXP)
                    for u, j in enumerate((j0, j1)):
                        kdiag = j - (nj - 4)
                        if kdiag >= 0:
                            dc = 512 * u + 128 * kdiag
                            nc.gpsimd.tensor_mul(
                                ex[:, dc:dc + 128], ex[:, dc:dc + 128],
                                mask_s[:])
                        if use_pad_mask:
                            c0 = j_c0(j)
                            nc.vector.tensor_scalar_mul(
                                ex[:, 512 * u + c0:512 * (u + 1)],
                                ex[:, 512 * u + c0:512 * (u + 1)],
                                pad_s[:, b * 16 + j:b * 16 + j + 1])
                    exs[(h, p)] = ex

                def emit_pv(h, p):
                    ex = exs.pop((h, p))
                    for u in range(2):
                        j = 2 * p + u
                        c0 = j_c0(j)
                        nc.tensor.matmul(
                            cps[h][:, c0:512],
                            va[:, j * 132 + 66 * h:j * 132 + 66 * h + 65],
                            ex[:, 512 * u + c0:512 * (u + 1)],
                            start=(j == 0), stop=(j == nj - 1))

                def fill(k):
                    if filler is not None:
                        for _ in range(k):
                            if not filler():
                                return

                emit_sc(0, 0)
                emit_sc(1, 0)
                for p in range(P):
                    emit_exp(0, p)
                    if p + 1 < P:
                        emit_sc(0, p + 1)
                    fill(1)
                    emit_pv(0, p)
                    emit_exp(1, p)
                    if p + 1 < P:
                        emit_sc(1, p + 1)
                    fill(1)
                    emit_pv(1, p)

                # normalize rows 0..63 by denominator row 64.  Engines
                # cannot replicate a row across partitions, but the PE can:
                # bcast[64h:64h+64, :] = ones64^T @ recip_row_h (K=1 matmul)
                # into a score-pool PSUM tile.  No DMA round-trip, so ctxt
                # is ready ~2us after the last PV and nothing downstream
                # ever waits on a multi-us broadcast chain.
                recs = [normtmp.tile([1, 512], BF16, tag=f"rec{h}",
                                     name=f"rec{b}{i}{h}")
                        for h in range(HPC)]
                with nc.allow_low_precision(reason="bf16 softmax denom"):
                    for h in range(HPC):
                        nc.vector.reciprocal(recs[h][:], cps[h][64:65, :])
                fill(2)
                bcp = psQ.tile([128, 512], F32, tag="proj",
                               name=f"bcp{b}{i}")
                for h in range(HPC):
                    nc.tensor.matmul(bcp[64 * h:64 * (h + 1), :],
                                     ones64[:], recs[h][:],
                                     start=True, stop=True)
                # engines cannot read two PSUM operands in one op: stage
                # the broadcast rows through SBUF
                bcs = normtmp.tile([128, 512], F32, tag="bcs",
                                   name=f"bcs{b}{i}")
                nc.vector.tensor_copy(bcs[:], bcp[:])
                for h in range(HPC):
                    hh = h * Dh
                    nc.vector.tensor_mul(ctxt[n][hh:hh + Dh, :],
                                         cps[h][0:64, :],
                                         bcs[64 * h:64 * (h + 1), :])

            # ---- out-projection as filler quanta --------------------------
            # One (e, chunk) matmul + eviction per quantum; the outT DMA
            # is issued one quantum LATE on the Activation HWDGE ring so
            # its eviction wait is already satisfied at issue time.
            ev_alt = [0]
            cqueue = []
            cstate = {"dma": None}

            def c_quantum(act_ok=False):
                st = cstate
                if st["dma"] is not None:
                    st["dma"]()
                    st["dma"] = None
                if not cqueue:
                    return False
                e, n2 = cqueue.pop(0)
                ps = psQ.tile([128, 512], F32, tag="proj",
                              name=f"op{e}_{n2}")
                nc.tensor.matmul(ps[:], wo_s[:, e * 128:(e + 1) * 128],
                                 ctxt[n2][:, :], start=True, stop=True)
                ev = outev.tile([128, 512], BF16, tag="ev",
                                name=f"ev{e}_{n2}")
                # Act evictions only outside attention windows: an Act
                # evict queued between exps can transitively stall the
                # whole exp stream on PE-side c-matmul dependencies.
                if act_ok and ev_alt[0] % 2 == 1:
                    nc.scalar.activation(ev[:], ps[:], IDN)
                else:
                    nc.vector.tensor_copy(ev[:], ps[:])
                ev_alt[0] += 1

                def dma(ev=ev, e=e, n2=n2):
                    nc.sync.dma_start(
                        out=outT[e * 128:(e + 1) * 128,
                                 n2 * 512:(n2 + 1) * 512],
                        in_=ev[:])
                st["dma"] = dma
                return True

            def drain_c(k):
                for _ in range(k):
                    if not c_quantum(act_ok=True):
                        return

            # ---- schedule -------------------------------------------------
            load_wplane(0)
            load_wplane(1)
            # chunk 0's x load is split in half (k-pairs 0-1, then 2-3) so
            # the first projection matmuls start ~1.4us earlier; the inner
            # 512B contiguous runs keep full DMA efficiency.
            xt0 = xpool.tile([128, 16, 512], F8, tag="xt", name="xt0")
            nc.sync.dma_start(out=xt0[:, 0:8, :], in_=x8[:, 0:8, 0:512])
            for _k in range(2, 6):
                load_wplane(_k)
            nc.sync.dma_start(out=xt0[:, 8:16, :], in_=x8[:, 8:16, 0:512])
            xt_tiles[0] = xt0
            emit_late_consts()
            emit_a_chunk(0)
            emit_a_chunk(1)
            flush_vt()
            emit_b_pair(0, 0)
            cqueue.extend((e, 0) for e in range(8))
            emit_a_chunk(2)
            drain_c(2)
            emit_b_pair(0, 1, filler=c_quantum)
            cqueue.extend((e, 1) for e in range(8))
            emit_a_chunk(3)
            drain_c(6)
            emit_b_pair(0, 2, filler=c_quantum)
            cqueue.extend((e, 2) for e in range(8))
            emit_a_chunk(4)
            drain_c(6)
            emit_b_pair(0, 3, filler=c_quantum)
            cqueue.extend((e, 3) for e in range(8))
            emit_a_chunk(5)
            drain_c(6)
            emit_b_pair(1, 0, filler=c_quantum)
            cqueue.extend((e, 4) for e in range(8))
            emit_a_chunk(6)
            drain_c(6)
            emit_b_pair(1, 1, filler=c_quantum)
            cqueue.extend((e, 5) for e in range(8))
            emit_a_chunk(7)
            drain_c(6)
            emit_b_pair(1, 2, filler=c_quantum)
            cqueue.extend((e, 6) for e in range(8))
            flush_vt()
            drain_c(6)
            emit_b_pair(1, 3, filler=c_quantum)
            cqueue.extend((e, 7) for e in range(8))
            drain_c(10 ** 6)
            if cstate["dma"] is not None:
                cstate["dma"]()
                cstate["dma"] = None
    return nc


_NC_CACHE = {}


def _get_nc(use_pad_mask: bool):
    if use_pad_mask not in _NC_CACHE:
        _NC_CACHE[use_pad_mask] = _build_nc(use_pad_mask)
    return _NC_CACHE[use_pad_mask]


def _host_inputs(x, attention_mask, qkv_w, qkv_b, out_w, use_pad_mask):
    import ml_dtypes
    E4 = ml_dtypes.float8_e4m3fn
    BF = ml_dtypes.bfloat16

    x = np.asarray(x, dtype=np.float32)
    qkv_w = np.asarray(qkv_w, dtype=np.float32)
    qkv_b = np.asarray(qkv_b, dtype=np.float32)
    out_w = np.asarray(out_w, dtype=np.float32)

    # x -> [128, (j, hl, s), NT] fp8 with hi/lo planes
    xT = x.reshape(NT, DIM).T                                # [1024, NT]
    xk = xT.reshape(4, 2, 128, NT).transpose(2, 0, 1, 3)     # [128,4,2,NT]
    xh = xk.astype(E4)
    xl = (xk - xh.astype(np.float32)).astype(E4)
    x8 = np.empty((128, 4, 2, 2, NT), dtype=E4)
    x8[:, :, 0, :, :] = xh
    x8[:, :, 1, :, :] = xl
    x8 = np.ascontiguousarray(x8.reshape(128, 16, NT))

    C, S = _rope_tables()
    ropecs = np.ascontiguousarray(
        np.concatenate([C, S], axis=1).astype(BF))
    perm = _row_perm()
    tri = (np.arange(128)[:, None] <= np.arange(128)[None, :])
    maskt = np.ascontiguousarray(tri.astype(BF))

    def pack_w(w_c):
        """[128 rows, 1024 K] -> hi/lo fp8 [128p, 4 kpair, 2 slot, 128 row]:
        element (p, j, s, m) = scaled W[m, 256j + 128s + p]."""
        ws = w_c * WSCALE
        wt = ws.T.reshape(4, 2, 128, RPC).transpose(2, 0, 1, 3)
        hi = wt.astype(E4)
        lo = (wt - hi.astype(np.float32)).astype(E4)
        return hi, lo

    in_maps = []
    for c in range(NCORES):
        heads = [HPC * c + h for h in range(HPC)]
        qrows = np.concatenate([h * Dh + perm for h in heads])
        vrows = np.concatenate([h * Dh + np.arange(Dh) for h in heads])
        wq_hi, wq_lo = pack_w(qkv_w[qrows, :] * SCALE)
        wk_hi, wk_lo = pack_w(qkv_w[DIM + qrows, :])
        wv_hi, wv_lo = pack_w(qkv_w[2 * DIM + vrows, :])
        wcat = np.ascontiguousarray(np.stack(
            [wq_hi, wq_lo, wk_hi, wk_lo, wv_hi, wv_lo], axis=1))
        bcat = np.ascontiguousarray(np.stack(
            [qkv_b[qrows] * SCALE, qkv_b[DIM + qrows],
             qkv_b[2 * DIM + vrows]], axis=1).astype(np.float32))
        m = {
            "x8": x8,
            "wcat": wcat,
            "bcat": bcat,
            "ropecs": ropecs,
            "maskt": maskt,
            "wo": np.ascontiguousarray(
                out_w[:, c * RPC:(c + 1) * RPC].T.astype(BF)),
        }
        if use_pad_mask:
            pad = np.asarray(attention_mask, dtype=np.float32)
            m["padv"] = np.ascontiguousarray(
                pad.reshape(B, T // 128, 128).transpose(0, 2, 1))
        in_maps.append(m)
    return in_maps


def kernel(x, attention_mask, qkv_w, qkv_b, out_w, out_b):
    from concourse.bass_utils import run_bass_kernel_spmd

    use_pad_mask = not np.asarray(attention_mask).all()
    nc = _get_nc(use_pad_mask)
    in_maps = _host_inputs(x, attention_mask, qkv_w, qkv_b, out_w,
                           use_pad_mask)
    res = run_bass_kernel_spmd(nc, in_maps, list(range(NCORES)))
    acc = res.results[0]["outT"].astype(np.float32)
    for c in range(1, NCORES):
        acc = acc + res.results[c]["outT"].astype(np.float32)
    out = acc.T + np.asarray(out_b, dtype=np.float32)[None, :]
    return np.ascontiguousarray(out.reshape(B, T, DIM), dtype=np.float32)


# revision 42
# speedup vs baseline: 1.3265x; 1.0811x over previous
"""Trainium2 Bass kernel for a RoPE causal-attention layer.

Problem (hardcoded): B=2, T=2048, DIM=1024, H=16 heads, Dh=64, fp32 I/O.
  qkv = x @ qkv_w.T + qkv_b ; rope(q), rope(k) ; causal softmax(q k^T / 8) @ v
  out = ctx @ out_w.T + out_b

Sharding: tensor-parallel over heads - each of the 8 cores owns 2 heads
(qkv_w row-sharded, out_w column-sharded); per-core bf16 partial outputs
are summed on the host.  HW exec (cost-model timeline): 170.8us vs the
225.5us fp32r v1 baseline (1.32x); measured rel err 4.3e-3 vs 2e-2 gate.
Staging-pool depths (outev=5, qkvtmp=5) matter: ~13us came from breaking
an eviction-tile WAR chain (ev slot reuse waited outT DMA + 900ns sem
propagation, stalling out-proj filler matmuls at the in-order PE queue).

Design:
  * QKV projection on the PE in fp8-e4m3 DoubleRow perf mode (two K=128
    tiles summed per instruction at 0.5 cycles/column) with 3-term error
    compensation: W_hi*x_hi + W_hi*x_lo + W_lo*x_hi (hi = e4m3, lo = e4m3
    of the residual).  Weights pre-scaled by 256 to escape e4m3
    subnormals; the (1/256)*psum + bias descale fuses into the eviction.
  * All other matmuls (scores, PV, out-proj) in bf16; PSUM stays f32.
  * RoPE: rows are 16-interleaved (A0 B0 A1 B1 per head; A/B = halves of
    the de-interleaved pair layout) so the half-swap is a DVE
    stream_shuffle (lane +-16 inside each 32-partition group) instead of
    a PE permutation matmul.  cos/sin ship pre-permuted in bf16.
  * V transposes via DMA XBAR (SBUF->SBUF bf16) into a staging tile; two
    Pool copies split each 128-token block into the per-head PV operand
    layout [64ch + ones] (blocks of 132 cols), giving contiguous 65-col
    matmul operands and softmax denominators on PSUM row 64 for free.
  * Scores are transposed [s,t]; per (head, j-pair) they land in a
    [128,1024] PSUM tile so exp runs 1024 wide (diagonal pairs compute
    the second block full-width so exp stays one instruction; acausal
    columns are never read by PV).  Triangle masks multiply on Pool.
  * Softmax normalization without any DMA round trip: DVE reciprocal of
    the denominator rows (bf16), then a K=1 ones-column PE matmul
    broadcasts each reciprocal row across 64 partitions into PSUM; one
    DVE copy stages it to SBUF and two DVE muls write normalized bf16
    context.  Context is ready ~2us after the last PV, so out-projection
    filler work never blocks the in-order PE queue.
  * Engine placement (GPSIMD cannot touch PSUM): Act = exps + q/k/v bias
    descales + boundary evictions; DVE = shuffles, rope cos-mul/add,
    reciprocal, normalize, in-block evictions; Pool = rope sin-mul,
    triangle masks, V-split copies, memsets.
  * Out-projection is emitted as (e, chunk) filler quanta inside the
    attention emission and at block boundaries, with the outT DMA issued
    one quantum late so its eviction wait is satisfied at issue time.
  * DMA count minimized (fixed ~600ns HWDGE + SEQ cost per transfer):
    one fp8 x-chunk load per 512 tokens (prefetched a chunk ahead), one
    concatenated weight tensor (6 plane DMAs, just-in-time at startup),
    batched rope tables, deferred V-transposes.
"""

import sys

if "/opt/trn_rl_repo" not in sys.path:
    sys.path.insert(0, "/opt/trn_rl_repo")

import numpy as np

import concourse.bass as bass
import concourse.tile as tile
from concourse import mybir
from concourse.vector_clock import ScopedClock, VectorClock

B, T, DIM = 2, 2048, 1024
H, Dh = 16, 64
NCORES = 8
HPC = H // NCORES          # heads per core
NT = B * T                 # 4096 tokens
RPC = HPC * Dh             # 128 q/k/v rows per core
NQC = NT // 512            # 8 token chunks
SCALE = Dh ** -0.5
WSCALE = 256.0             # fp8 weight pre-scale (escape e4m3 subnormals)

F32 = mybir.dt.float32
BF16 = mybir.dt.bfloat16
F8 = mybir.dt.float8e4

SWAP16 = [(i + 16) % 32 for i in range(32)]


def _patch_tile_drain():
    """This container's walrus build allows at most ONE semaphore wait per
    instruction (setupSyncWait rejects more).  Two fixes:
      1. Tile's end-of-kernel drain accumulates one wait per live
         semaphore - split into one drain per semaphore.
      2. Any scheduled instruction that received >1 sem waits in stage 1B
         gets its extra waits hoisted onto same-engine NoOps emitted just
         before it during lowering."""
    if getattr(tile.TileContext, "_drain_patched", False):
        return

    def patched(self, tick_clock, wait_clock):
        vec = list(tick_clock.global_clock)
        nz = [(i, t) for i, t in enumerate(vec) if t > 0] or [(0, 0)]
        for i, t in nz:
            cv = [0] * len(vec)
            cv[i] = t
            d = self.nc.sync.drain()
            wait_clock.add_sem_waits(d.ins, ScopedClock({None: VectorClock(cv)}))
        self.nc.all_engine_barrier()
        popped = self.nc._tile_sem_poison_stack.pop()
        assert popped is self._sem_poison
        self.nc.clear_and_free_semaphores(list(self.sems.allocated().values()))
        self.nc.all_engine_barrier()

    tile.TileContext._drain_and_barrier = patched

    orig_cal = tile.TileContext._commit_and_lower

    def patched_cal(self, inst, original_block, old_bb_map, bb_to_exit_bb):
        si = getattr(inst, "sync_info", None)
        eng = getattr(inst, "engine", None)
        if si is not None and si.on_wait and eng in self.nc.engines:
            waits = list(si.on_wait)
            keep = 1
            if len(waits) > keep:
                for w in waits[: len(waits) - keep]:
                    nop = self.nc.engines[eng].nop(nofuse=True)
                    nop.ins.sync_info = mybir.SyncInfo(on_wait=[w], on_update=[])
                inst.sync_info = mybir.SyncInfo(
                    on_wait=waits[len(waits) - keep:],
                    on_update=list(si.on_update or []),
                )
        return orig_cal(self, inst, original_block, old_bb_map, bb_to_exit_bb)

    tile.TileContext._commit_and_lower = patched_cal
    tile.TileContext._drain_patched = True


def _deint_pos():
    """dp[p]: de-interleaved-layout position feeding final row p (64-row
    head).  Final blocks of 16: [A0, B0, A1, B1] with A = deint 0..31
    (pair element 0), B = deint 32..63 (element 1).  RoPE partners land
    +-16 apart inside each 32-partition group -> stream_shuffle SWAP16."""
    dp = np.empty(64, dtype=np.int64)
    dp[0:16] = np.arange(0, 16)
    dp[16:32] = np.arange(32, 48)
    dp[32:48] = np.arange(16, 32)
    dp[48:64] = np.arange(48, 64)
    return dp


def _row_perm():
    """packed_q[p] = q[perm[p]] within one head's 64 rows."""
    deint = np.empty(64, dtype=np.int64)
    deint[:32] = 2 * np.arange(32)
    deint[32:] = 2 * np.arange(32) + 1
    return deint[_deint_pos()]


def _rope_tables():
    """C, S [128, T] f32 in the final row layout (2 heads stacked)."""
    half = 32
    inv_freq = 1.0 / 10000.0 ** (np.arange(half, dtype=np.float64) / half)
    pair_freq = inv_freq[(2 * np.arange(half)) % half]
    ang = np.outer(pair_freq, np.arange(T, dtype=np.float64))
    c32, s32 = np.cos(ang), np.sin(ang)
    Cd = np.concatenate([c32, c32], axis=0)
    Sd = np.concatenate([-s32, s32], axis=0)
    dp = _deint_pos()
    C64, S64 = Cd[dp], Sd[dp]
    C = np.concatenate([C64, C64], axis=0)
    S = np.concatenate([S64, S64], axis=0)
    return C, S


def _build_nc(use_pad_mask: bool):
    _patch_tile_drain()
    nc = bass.Bass("TRN2", target_bir_lowering=False, debug=False,
                   num_devices=NCORES)

    MUL = mybir.AluOpType.mult
    ADD = mybir.AluOpType.add
    DR = mybir.MatmulPerfMode.DoubleRow
    EXP = mybir.ActivationFunctionType.Exp
    IDN = mybir.ActivationFunctionType.Identity

    # x8: (p, j kpair, hl, s slot, t) flattened to [128, 16, NT]:
    # col block 4*j + 2*hl + s.
    x8 = nc.dram_tensor("x8", [128, 16, NT], F8, kind="ExternalInput")
    # wcat: 6 weight planes (wqh wql wkh wkl wvh wvl), each (j, s, row):
    # [128, 6, 4, 2, 128]
    wcat = nc.dram_tensor("wcat", [128, 6, 4, 2, RPC], F8,
                          kind="ExternalInput")
    bcat = nc.dram_tensor("bcat", [RPC, 3], F32, kind="ExternalInput")
    ropecs = nc.dram_tensor("ropecs", [RPC, 2 * T], BF16,
                            kind="ExternalInput")
    maskt = nc.dram_tensor("maskt", [128, 128], BF16, kind="ExternalInput")
    wo = nc.dram_tensor("wo", [RPC, DIM], BF16, kind="ExternalInput")
    if use_pad_mask:
        padv = nc.dram_tensor("padv", [B, 128, T // 128], F32,
                              kind="ExternalInput")
    outT = nc.dram_tensor("outT", [DIM, NT], BF16, kind="ExternalOutput")

    with tile.TileContext(nc) as tc:
        with (
            tc.tile_pool(name="consts", bufs=1) as consts,
            tc.tile_pool(name="persist", bufs=1) as persist,
            tc.tile_pool(name="xpool", bufs=2) as xpool,
            tc.tile_pool(name="qkvtmp", bufs=5) as qkvtmp,
            tc.tile_pool(name="ropetmp", bufs=4) as ropetmp,
            tc.tile_pool(name="vspool", bufs=8) as vspool,
            tc.tile_pool(name="exppool", bufs=4) as exppool,
            tc.tile_pool(name="normtmp", bufs=2) as normtmp,
            tc.tile_pool(name="outev", bufs=5) as outev,
            tc.tile_pool(name="drampool", bufs=2, space="DRAM") as drampool,
            tc.tile_pool(name="psQ", bufs=2, space="PSUM") as psQ,
            tc.tile_pool(name="psS", bufs=2, space="PSUM") as psS,
            tc.tile_pool(name="psC", bufs=2, space="PSUM") as psC,
        ):
            # ---- constants: wcat ahead of x0; the rest are emitted by
            # emit_late_consts() between the first two A chunks so the x0
            # transfer isn't pushed out on the serial DMA path ------------
            w_s = consts.tile([128, 6, 4, 2, RPC], F8, tag="wcat")

            def load_wplane(k):
                nc.sync.dma_start(out=w_s[:, k], in_=wcat[:, k])
            b_s = consts.tile([RPC, 3], F32, tag="bcat")
            cs_s = consts.tile([RPC, 2 * T], BF16, tag="ropecs")
            mask_s = consts.tile([128, 128], BF16, tag="maskt")
            ones64 = consts.tile([1, 64], BF16, tag="ones64")
            nc.vector.memset(ones64[:], 1.0)
            wo_s = consts.tile([RPC, DIM], BF16, tag="wo")
            if use_pad_mask:
                pad_s = consts.tile([128, B * (T // 128)], F32, tag="padv")

            def emit_late_consts():
                nc.sync.dma_start(out=cs_s[:], in_=ropecs[:])
                nc.sync.dma_start(out=b_s[:], in_=bcat[:])
                nc.sync.dma_start(out=mask_s[:], in_=maskt[:])
                if use_pad_mask:
                    for b in range(B):
                        nc.sync.dma_start(
                            out=pad_s[:, b * 16:(b + 1) * 16], in_=padv[b])
                # wo on the SWDGE ring; its consumer is far downstream.
                nc.gpsimd.dma_start(out=wo_s[:], in_=wo[:])

            ropec_s = cs_s[:, 0:T]
            ropes_s = cs_s[:, T:2 * T]
            W = {"wqh": 0, "wql": 1, "wkh": 2, "wkl": 3, "wvh": 4, "wvl": 5}

            # ---- persistent activations ----------------------------------
            qrot = [persist.tile([RPC, 512], BF16, tag=f"qrot{n}",
                                 name=f"qrot{n}") for n in range(NQC)]
            krot = [persist.tile([RPC, 512], BF16, tag=f"krot{n}",
                                 name=f"krot{n}") for n in range(NQC)]
            ctxt = [persist.tile([RPC, 512], BF16, tag=f"ctxt{n}",
                                 name=f"ctxt{n}") for n in range(NQC)]
            # vaug block layout (132 cols per 128-token block):
            #   [h0 ch0..63, one0, pad, h1 ch0..63, one1, pad]
            # The XBAR transpose lands a 128-token block in a contiguous
            # staging tile; two Pool copies split it into the two heads'
            # slots so each PV operand is a contiguous 65-col slice
            # [64ch + ones] with context rows at partition base 0 and the
            # denominator on row 64 (HW requires partition bases 0/32/64).
            vaug = {}
            for b in range(B):
                va = persist.tile([128, 16 * 132], BF16, tag=f"vaug{b}")
                nc.gpsimd.memset(va[:], 1.0)
                vaug[b] = va

            def rope(raw, dst, tloc):
                sw = ropetmp.tile([128, 512], BF16, tag="sw")
                nc.vector.stream_shuffle(sw[:], raw[:], SWAP16)
                nc.vector.tensor_mul(dst, raw[:],
                                     ropec_s[:, tloc:tloc + 512])
                sw2 = ropetmp.tile([128, 512], BF16, tag="sw2")
                nc.gpsimd.tensor_mul(sw2[:], sw[:],
                                     ropes_s[:, tloc:tloc + 512])
                nc.vector.tensor_add(dst, dst, sw2[:])

            pending_vt = []
            pending_rope = []

            def flush_vt():
                while pending_vt:
                    pending_vt.pop(0)()

            def flush_rope():
                # ropes emitted one slot after their bias so the DVE
                # sequencer never blocks waiting on the Act bias op
                while pending_rope:
                    raw, dst, tloc = pending_rope.pop(0)
                    rope(raw, dst, tloc)

            xt_tiles = {}

            def load_x(n):
                xt = xpool.tile([128, 16, 512], F8, tag="xt", name=f"xt{n}")
                nc.sync.dma_start(out=xt[:],
                                  in_=x8[:, :, n * 512:(n + 1) * 512])
                xt_tiles[n] = xt

            def emit_a_chunk(n):
                """QKV fp8 projection + RoPE + V DMA-transpose for one
                512-token chunk.  x for chunk n+1 is prefetched first;
                deferred work from earlier chunks (denominator broadcast,
                V transposes, normalize muls) is flushed at points where
                its waits are already satisfied."""
                if n + 1 < NQC:
                    load_x(n + 1)
                flush_rope()
                flush_vt()
                t0 = n * 512
                tloc = t0 % T
                b = t0 // T
                xt = xt_tiles.pop(n)

                def xs(j, hl):
                    return xt[:, 4 * j + 2 * hl:4 * j + 2 * hl + 2, :]

                def project(hi, lo, bi, dst, critical):
                    ps = psQ.tile([128, 512], F32, tag="proj")
                    terms = [(hi, 0), (hi, 1), (lo, 0)]
                    for ti, (w, hl) in enumerate(terms):
                        for j in range(4):
                            nc.tensor.matmul(
                                ps[:], w_s[:, W[w], j], xs(j, hl),
                                start=(ti == 0 and j == 0),
                                stop=(ti == 2 and j == 3),
                                perf_mode=DR)
                    # (1/256)*psum + bias, f32 PSUM -> bf16 SBUF.  GPSIMD
                    # has no PSUM access, so this is Act (q/k: feeds RoPE,
                    # short queue) or DVE (v: consumed late).
                    if critical:
                        nc.scalar.activation(dst[:], ps[:], IDN,
                                             bias=b_s[:, bi:bi + 1],
                                             scale=1.0 / WSCALE)
                    else:
                        nc.vector.tensor_scalar(
                            out=dst[:], in0=ps[:], scalar1=1.0 / WSCALE,
                            scalar2=b_s[:, bi:bi + 1], op0=MUL, op1=ADD)

                qraw = qkvtmp.tile([128, 512], BF16, tag="qraw")
                project("wqh", "wql", 0, qraw, critical=True)
                kraw = qkvtmp.tile([128, 512], BF16, tag="kraw")
                project("wkh", "wkl", 1, kraw, critical=True)
                pending_rope.append((qraw, qrot[n][:, :], tloc))
                vraw = qkvtmp.tile([128, 512], BF16, tag="vraw",
                                   name=f"vraw{n}")
                project("wvh", "wvl", 2, vraw, critical=False)
                pending_rope.append((kraw, krot[n][:, :], tloc))
                va = vaug[b]

                def vtrans():
                    for m in range(4):
                        blk = (tloc + m * 128) // 128
                        vs = vspool.tile([128, 128], BF16, tag="vstage",
                                         name=f"vs{n}_{m}")
                        nc.sync.dma_start_transpose(
                            vs[:], vraw[:, m * 128:(m + 1) * 128])
                        for h in range(HPC):
                            nc.gpsimd.tensor_copy(
                                va[:, blk * 132 + 66 * h:
                                   blk * 132 + 66 * h + 64],
                                vs[:, 64 * h:64 * (h + 1)])
                pending_vt.append(vtrans)

            def emit_b_pair(b, i, filler=None):
                """Flash attention for one (batch, 512-query block), both
                heads software-pipelined; j-blocks pair into [128,1024]
                PSUM tiles per head, heads alternating, so exp runs 1024
                wide and each exp has a full other-head cycle to
                complete."""
                nj = 4 * (i + 1)
                P = nj // 2
                n = 4 * b + i
                cps = {}
                for h in range(HPC):
                    cps[h] = psC.tile([65, 512], F32, tag="cps",
                                      name=f"cps{b}{h}{i}")
                va = vaug[b]
                grp = {}
                exs = {}

                def j_c0(j):
                    kdiag = j - (nj - 4)
                    return 128 * kdiag if kdiag > 0 else 0

                def emit_sc(h, p):
                    hh = h * Dh
                    g = psS.tile([128, 1024], F32, tag="grp",
                                 name=f"g{b}{i}{h}{p}")
                    for u in range(2):
                        j = 2 * p + u
                        kchunk = (b * T + j * 128) // 512
                        koff = (j * 128) % 512
                        # u=1 computes full width even on diagonal blocks
                        # so exp can run as one contiguous instruction;
                        # the acausal columns are never read by PV.
                        c0 = j_c0(j) if u == 0 else 0
                        nc.tensor.matmul(
                            g[:, 512 * u + c0:512 * (u + 1)],
                            krot[kchunk][hh:hh + Dh, koff:koff + 128],
                            qrot[n][hh:hh + Dh, c0:512],
                            start=True, stop=True)
                    grp[(h, p)] = g

                def emit_exp(h, p):
                    g = grp.pop((h, p))
                    ex = exppool.tile([128, 1024], BF16, tag="ex")
                    j0, j1 = 2 * p, 2 * p + 1
                    c0 = j_c0(j0)
                    nc.scalar.activation(ex[:, c0:1024], g[:, c0:1024], EXP)
                    for u, j in enumerate((j0, j1)):
                        kdiag = j - (nj - 4)
                        if kdiag >= 0:
                            dc = 512 * u + 128 * kdiag
                            nc.gpsimd.tensor_mul(
                                ex[:, dc:dc + 128], ex[:, dc:dc + 128],
                                mask_s[:])
                        if use_pad_mask:
                            c0 = j_c0(j)
                            nc.vector.tensor_scalar_mul(
                                ex[:, 512 * u + c0:512 * (u + 1)],
                                ex[:, 512 * u + c0:512 * (u + 1)],
                                pad_s[:, b * 16 + j:b * 16 + j + 1])
                    exs[(h, p)] = ex

                def emit_pv(h, p):
                    ex = exs.pop((h, p))
                    for u in range(2):
                        j = 2 * p + u
                        c0 = j_c0(j)
                        nc.tensor.matmul(
                            cps[h][:, c0:512],
                            va[:, j * 132 + 66 * h:j * 132 + 66 * h + 65],
                            ex[:, 512 * u + c0:512 * (u + 1)],
                            start=(j == 0), stop=(j == nj - 1))

                def fill(k):
                    if filler is not None:
                        for _ in range(k):
                            if not filler():
                                return

                emit_sc(0, 0)
                emit_sc(1, 0)
                for p in range(P):
                    emit_exp(0, p)
                    if p + 1 < P:
                        emit_sc(0, p + 1)
                    fill(1)
                    emit_pv(0, p)
                    emit_exp(1, p)
                    if p + 1 < P:
                        emit_sc(1, p + 1)
                    fill(1)
                    emit_pv(1, p)

                # normalize rows 0..63 by denominator row 64.  Engines
                # cannot replicate a row across partitions, but the PE can:
                # bcast[64h:64h+64, :] = ones64^T @ recip_row_h (K=1 matmul)
                # into a score-pool PSUM tile.  No DMA round-trip, so ctxt
                # is ready ~2us after the last PV and nothing downstream
                # ever waits on a multi-us broadcast chain.
                recs = [normtmp.tile([1, 512], BF16, tag=f"rec{h}",
                                     name=f"rec{b}{i}{h}")
                        for h in range(HPC)]
                with nc.allow_low_precision(reason="bf16 softmax denom"):
                    for h in range(HPC):
                        nc.vector.reciprocal(recs[h][:], cps[h][64:65, :])
                fill(2)
                bcp = psQ.tile([128, 512], F32, tag="proj",
                               name=f"bcp{b}{i}")
                for h in range(HPC):
                    nc.tensor.matmul(bcp[64 * h:64 * (h + 1), :],
                                     ones64[:], recs[h][:],
                                     start=True, stop=True)
                # engines cannot read two PSUM operands in one op: stage
                # the broadcast rows through SBUF
                bcs = normtmp.tile([128, 512], F32, tag="bcs",
                                   name=f"bcs{b}{i}")
                nc.vector.tensor_copy(bcs[:], bcp[:])
                for h in range(HPC):
                    hh = h * Dh
                    nc.vector.tensor_mul(ctxt[n][hh:hh + Dh, :],
                                         cps[h][0:64, :],
                                         bcs[64 * h:64 * (h + 1), :])

            # ---- out-projection as filler quanta --------------------------
            # One (e, chunk) matmul + eviction per quantum; the outT DMA
            # is issued one quantum LATE on the Activation HWDGE ring so
            # its eviction wait is already satisfied at issue time.
            ev_alt = [0]
            cqueue = []
            cstate = {"dma": None}

            def c_quantum(act_ok=False):
                st = cstate
                if st["dma"] is not None:
                    st["dma"]()
                    st["dma"] = None
                if not cqueue:
                    return False
                e, n2 = cqueue.pop(0)
                ps = psQ.tile([128, 512], F32, tag="proj",
                              name=f"op{e}_{n2}")
                nc.tensor.matmul(ps[:], wo_s[:, e * 128:(e + 1) * 128],
                                 ctxt[n2][:, :], start=True, stop=True)
                ev = outev.tile([128, 512], BF16, tag="ev",
                                name=f"ev{e}_{n2}")
                # Act evictions only outside attention windows: an Act
                # evict queued between exps can transitively stall the
                # whole exp stream on PE-side c-matmul dependencies.
                if act_ok and ev_alt[0] % 2 == 1:
                    nc.scalar.activation(ev[:], ps[:], IDN)
                else:
                    nc.vector.tensor_copy(ev[:], ps[:])
                ev_alt[0] += 1

                def dma(ev=ev, e=e, n2=n2):
                    nc.sync.dma_start(
                        out=outT[e * 128:(e + 1) * 128,
                                 n2 * 512:(n2 + 1) * 512],
                        in_=ev[:])
                st["dma"] = dma
                return True

            def drain_c(k):
                for _ in range(k):
                    if not c_quantum(act_ok=True):
                        return

            # ---- schedule -------------------------------------------------
            load_wplane(0)
            load_wplane(1)
            # chunk 0's x load is split in half (k-pairs 0-1, then 2-3) so
            # the first projection matmuls start ~1.4us earlier; the inner
            # 512B contiguous runs keep full DMA efficiency.
            xt0 = xpool.tile([128, 16, 512], F8, tag="xt", name="xt0")
            nc.sync.dma_start(out=xt0[:, 0:8, :], in_=x8[:, 0:8, 0:512])
            for _k in range(2, 6):
                load_wplane(_k)
            nc.sync.dma_start(out=xt0[:, 8:16, :], in_=x8[:, 8:16, 0:512])
            xt_tiles[0] = xt0
            emit_late_consts()
            emit_a_chunk(0)
            emit_a_chunk(1)
            flush_vt()
            emit_b_pair(0, 0)
            cqueue.extend((e, 0) for e in range(8))
            emit_a_chunk(2)
            drain_c(2)
            emit_b_pair(0, 1, filler=c_quantum)
            cqueue.extend((e, 1) for e in range(8))
            emit_a_chunk(3)
            drain_c(6)
            emit_b_pair(0, 2, filler=c_quantum)
            cqueue.extend((e, 2) for e in range(8))
            emit_a_chunk(4)
            drain_c(6)
            emit_b_pair(0, 3, filler=c_quantum)
            cqueue.extend((e, 3) for e in range(8))
            emit_a_chunk(5)
            drain_c(6)
            emit_b_pair(1, 0, filler=c_quantum)
            cqueue.extend((e, 4) for e in range(8))
            emit_a_chunk(6)
            drain_c(6)
            emit_b_pair(1, 1, filler=c_quantum)
            cqueue.extend((e, 5) for e in range(8))
            emit_a_chunk(7)
            drain_c(6)
            emit_b_pair(1, 2, filler=c_quantum)
            cqueue.extend((e, 6) for e in range(8))
            flush_rope()
            flush_vt()
            drain_c(2)
            emit_b_pair(1, 3, filler=c_quantum)
            cqueue.extend((e, 7) for e in range(8))
            drain_c(10 ** 6)
            if cstate["dma"] is not None:
                cstate["dma"]()
                cstate["dma"] = None
    return nc


_NC_CACHE = {}


def _get_nc(use_pad_mask: bool):
    if use_pad_mask not in _NC_CACHE:
        _NC_CACHE[use_pad_mask] = _build_nc(use_pad_mask)
    return _NC_CACHE[use_pad_mask]


def _host_inputs(x, attention_mask, qkv_w, qkv_b, out_w, use_pad_mask):
    import ml_dtypes
    E4 = ml_dtypes.float8_e4m3fn
    BF = ml_dtypes.bfloat16

    x = np.asarray(x, dtype=np.float32)
    qkv_w = np.asarray(qkv_w, dtype=np.float32)
    qkv_b = np.asarray(qkv_b, dtype=np.float32)
    out_w = np.asarray(out_w, dtype=np.float32)

    # x -> [128, (j, hl, s), NT] fp8 with hi/lo planes
    xT = x.reshape(NT, DIM).T                                # [1024, NT]
    xk = xT.reshape(4, 2, 128, NT).transpose(2, 0, 1, 3)     # [128,4,2,NT]
    xh = xk.astype(E4)
    xl = (xk - xh.astype(np.float32)).astype(E4)
    x8 = np.empty((128, 4, 2, 2, NT), dtype=E4)
    x8[:, :, 0, :, :] = xh
    x8[:, :, 1, :, :] = xl
    x8 = np.ascontiguousarray(x8.reshape(128, 16, NT))

    C, S = _rope_tables()
    ropecs = np.ascontiguousarray(
        np.concatenate([C, S], axis=1).astype(BF))
    perm = _row_perm()
    tri = (np.arange(128)[:, None] <= np.arange(128)[None, :])
    maskt = np.ascontiguousarray(tri.astype(BF))

    def pack_w(w_c):
        """[128 rows, 1024 K] -> hi/lo fp8 [128p, 4 kpair, 2 slot, 128 row]:
        element (p, j, s, m) = scaled W[m, 256j + 128s + p]."""
        ws = w_c * WSCALE
        wt = ws.T.reshape(4, 2, 128, RPC).transpose(2, 0, 1, 3)
        hi = wt.astype(E4)
        lo = (wt - hi.astype(np.float32)).astype(E4)
        return hi, lo

    in_maps = []
    for c in range(NCORES):
        heads = [HPC * c + h for h in range(HPC)]
        qrows = np.concatenate([h * Dh + perm for h in heads])
        vrows = np.concatenate([h * Dh + np.arange(Dh) for h in heads])
        wq_hi, wq_lo = pack_w(qkv_w[qrows, :] * SCALE)
        wk_hi, wk_lo = pack_w(qkv_w[DIM + qrows, :])
        wv_hi, wv_lo = pack_w(qkv_w[2 * DIM + vrows, :])
        wcat = np.ascontiguousarray(np.stack(
            [wq_hi, wq_lo, wk_hi, wk_lo, wv_hi, wv_lo], axis=1))
        bcat = np.ascontiguousarray(np.stack(
            [qkv_b[qrows] * SCALE, qkv_b[DIM + qrows],
             qkv_b[2 * DIM + vrows]], axis=1).astype(np.float32))
        m = {
            "x8": x8,
            "wcat": wcat,
            "bcat": bcat,
            "ropecs": ropecs,
            "maskt": maskt,
            "wo": np.ascontiguousarray(
                out_w[:, c * RPC:(c + 1) * RPC].T.astype(BF)),
        }
        if use_pad_mask:
            pad = np.asarray(attention_mask, dtype=np.float32)
            m["padv"] = np.ascontiguousarray(
                pad.reshape(B, T // 128, 128).transpose(0, 2, 1))
        in_maps.append(m)
    return in_maps


def kernel(x, attention_mask, qkv_w, qkv_b, out_w, out_b):
    from concourse.bass_utils import run_bass_kernel_spmd

    use_pad_mask = not np.asarray(attention_mask).all()
    nc = _get_nc(use_pad_mask)
    in_maps = _host_inputs(x, attention_mask, qkv_w, qkv_b, out_w,
                           use_pad_mask)
    res = run_bass_kernel_spmd(nc, in_maps, list(range(NCORES)))
    acc = res.results[0]["outT"].astype(np.float32)
    for c in range(1, NCORES):
        acc = acc + res.results[c]["outT"].astype(np.float32)
    out = acc.T + np.asarray(out_b, dtype=np.float32)[None, :]
    return np.ascontiguousarray(out.reshape(B, T, DIM), dtype=np.float32)
